# revision 39
# baseline (speedup 1.0000x reference)
"""Cross multi-head attention on 8 Trainium2 NeuronCores.

Sharding: tensor-parallel over heads x data-parallel over batch.
Core c handles batch b = c//2 and head-half hh = c%2 (8 of 16 heads),
full S=2048 queries, full T=2048 keys. No K/V recompute (the old
batch x seq-half layout projected K/V twice per batch). Each core
emits a PARTIAL output (its 8 heads' concat @ Wo slice, no bias);
the host sums the two partials per batch and adds bo. That host-side
reduce is the tensor-parallel unshard for the contraction split of
combo_linear.

Per-core kernel (T-major layout, no on-chip transposes; host
pre-transposes x/y and pre-packs weights):
  QT[hp]  [128,S]  = Wq2[hp].T @ xT      (head-pair packed: rows 0:64 head a,
  KT[hp]  [128,T]  = Wk2[hp].T @ yT       rows 64:128 head b; bias fused into
                                          the PSUM->SBUF copy on DVE)
  V'      [t,h,65] = yT.T @ Wv_cat | 1   (natural layout + ones column;
                                          projected per t-tile just-in-time
                                          inside the first attention pass)
  scoresT [t,q]    = KT_h.T @ QT_h       (K=64 contraction; two heads truly
                                          concurrent via PE row tiling)
  expT    = exp(scoresT * 0.125)         (one ACT op per psum pair tile)
  oT'     += V'_h.T @ expT               (M=65: row 64 accumulates the softmax
                                          denominator for free)
  oT      = oT'[0:64] * bcast(1/oT'[64]) (deferred normalization; fast
                                          approximate reciprocal on DVE)
  partial = concatT.T @ Wo               (per-qc, interleaved into the last
                                          head-pair's attention)
Matmul inputs bf16 (fp32 PSUM accumulation), softmax in fp32.
"""

import numpy as np

B, S, T, E, H, D = 4, 2048, 2048, 1024, 16, 64
N_CORES = 8

_compiled = {}


def _dt():
    from concourse import mybir

    return mybir.dt


def _mybir():
    from concourse import mybir

    return mybir


def build_program(n_hp=4, s_loc=2048, t_len=2048, n_et=8, dve_exp_tts=(), debug_taps=False):
    """Emit the per-core bass program.

    n_hp: head pairs on this core (heads = 2*n_hp), s_loc: query rows,
    t_len: key rows, n_et: contraction tiles (emb dim = 128*n_et).
    dve_exp_tts: set of tt indices whose softmax exp runs on DVE via the
    Schraudolph bit trick instead of ACT (load balancing knob).
    """
    import concourse.tile as tile
    from concourse import bacc

    dt = _dt()
    bf16 = dt.bfloat16
    f32 = dt.float32

    e_dim = 128 * n_et
    c_dim = 64 * 2 * n_hp  # concat dim of this core's heads
    n_h = 2 * n_hp
    n_tt = t_len // 128  # key tiles
    qch = min(512, s_loc)  # query chunk width
    tch = min(512, t_len)
    ech = min(512, e_dim)
    n_qc = s_loc // qch  # query chunks for attention
    n_st = s_loc // 128  # output row tiles
    n_ec = e_dim // ech  # output col chunks
    n_tc = t_len // tch

    nc = bacc.Bacc("TRN2", target_bir_lowering=False, debug=False)

    # ---- DRAM I/O (host provides these layouts directly) ----
    xT = nc.dram_tensor("xT", [128, n_et, s_loc], bf16, kind="ExternalInput").ap()
    yT = nc.dram_tensor("yT", [128, n_et, t_len], bf16, kind="ExternalInput").ap()
    wq2 = nc.dram_tensor("wq2", [128, n_hp, n_et, 128], bf16, kind="ExternalInput").ap()
    wk2 = nc.dram_tensor("wk2", [128, n_hp, n_et, 128], bf16, kind="ExternalInput").ap()
    wv = nc.dram_tensor("wv", [128, n_et, c_dim], bf16, kind="ExternalInput").ap()
    wo = nc.dram_tensor("wo", [128, n_hp, e_dim], bf16, kind="ExternalInput").ap()
    bqc = nc.dram_tensor("bqc", [128, n_hp], f32, kind="ExternalInput").ap()
    bkc = nc.dram_tensor("bkc", [128, n_hp], f32, kind="ExternalInput").ap()
    bvc = nc.dram_tensor("bvc", [1, c_dim], bf16, kind="ExternalInput").ap()
    out = nc.dram_tensor("out", [s_loc, e_dim], f32, kind="ExternalOutput").ap()

    from contextlib import ExitStack

    dbg = {}
    if debug_taps:
        dbg["qt0"] = nc.dram_tensor("dbg_qt0", [128, s_loc], dt.bfloat16, kind="ExternalOutput").ap()
        dbg["kt0"] = nc.dram_tensor("dbg_kt0", [128, t_len], dt.bfloat16, kind="ExternalOutput").ap()
        dbg["v"] = nc.dram_tensor("dbg_v", [128, t_len // 128, 2 * n_hp, 65], dt.bfloat16, kind="ExternalOutput").ap()
        dbg["oT"] = nc.dram_tensor("dbg_oT", [128, n_hp, s_loc], dt.bfloat16, kind="ExternalOutput").ap()
        dbg["osc0"] = nc.dram_tensor("dbg_osc0", [64, min(512, s_loc)], dt.bfloat16, kind="ExternalOutput").ap()
        dbg["den0"] = nc.dram_tensor("dbg_den0", [1, min(512, s_loc)], f32, kind="ExternalOutput").ap()
        dbg["rcp0"] = nc.dram_tensor("dbg_rcp0", [1, min(512, s_loc)], f32, kind="ExternalOutput").ap()
        dbg["exp0"] = nc.dram_tensor("dbg_exp0", [128, 2, min(512, s_loc)], dt.bfloat16, kind="ExternalOutput").ap()

    with tile.TileContext(nc) as tc, ExitStack() as ctx:
        consts = ctx.enter_context(tc.tile_pool(name="consts", bufs=1))
        scr_pool = ctx.enter_context(tc.tile_pool(name="scr", bufs=2, space="DRAM"))
        qt_pool = ctx.enter_context(tc.tile_pool(name="qt", bufs=2))
        kt_pool = ctx.enter_context(tc.tile_pool(name="kt", bufs=2))
        exp_pool = ctx.enter_context(tc.tile_pool(name="expp", bufs=5))
        schr_pool = ctx.enter_context(tc.tile_pool(name="schr", bufs=1))
        osc_pool = ctx.enter_context(tc.tile_pool(name="osc", bufs=8))
        rcp_pool = ctx.enter_context(tc.tile_pool(name="rcp", bufs=4))
        den_pool = ctx.enter_context(tc.tile_pool(name="den", bufs=2))
        rbc_pool = ctx.enter_context(tc.tile_pool(name="rbc", bufs=4))
        osb_pool = ctx.enter_context(tc.tile_pool(name="osb", bufs=2))
        sc_ps = ctx.enter_context(tc.tile_pool(name="scps", bufs=2, space="PSUM"))
        acc_ps = ctx.enter_context(tc.tile_pool(name="accps", bufs=2, space="PSUM"))
        o_ps_pool = ctx.enter_context(tc.tile_pool(name="ops", bufs=2, space="PSUM"))

        # ---- resident tiles ----
        xT_sb = consts.tile([128, n_et, s_loc], bf16)
        yT_sb = consts.tile([128, n_et, t_len], bf16)
        wq_sb = consts.tile([128, n_hp, n_et, 128], bf16)
        wk_sb = consts.tile([128, n_hp, n_et, 128], bf16)
        wv_sb = consts.tile([128, n_et, c_dim], bf16)
        wo_sb = consts.tile([128, n_hp, e_dim], bf16)
        bqc_sb = consts.tile([128, n_hp], f32)
        bkc_sb = consts.tile([128, n_hp], f32)
        bv_sb = consts.tile([1, c_dim], bf16)

        # DMA ordering: the critical path to the first attention tile is
        # yT chunk0 -> K-proj hp0 chunk0 -> scores (plus wv for the JIT
        # V-projection). 512-col chunks keep region deps fine-grained so
        # compute starts as soon as the first column chunk of each et row
        # lands; spread across the three DMA-capable engine queues.
        bv_bc = consts.tile([128, c_dim], bf16)
        nc.sync.dma_start(out=wk_sb[:, 0, :, :], in_=wk2[:, 0, :, :])
        nc.gpsimd.dma_start(out=wq_sb[:, 0, :, :], in_=wq2[:, 0, :, :])
        # the scalar (ACT) queue gets ONLY early small transfers: every
        # dma_start on it occupies the ACT instruction FIFO, so bulk loads
        # there would stall the exp pipeline behind the DMA ring
        nc.scalar.dma_start(out=bqc_sb, in_=bqc)
        nc.scalar.dma_start(out=bkc_sb, in_=bkc)
        nc.scalar.dma_start(out=bv_sb, in_=bvc)
        # wv gates the JIT V-projection right behind the first K/Q chunks
        nc.sync.dma_start(out=wv_sb[:, :, 0 : c_dim // 2], in_=wv[:, :, 0 : c_dim // 2])
        nc.gpsimd.dma_start(out=wv_sb[:, :, c_dim // 2 :], in_=wv[:, :, c_dim // 2 :])
        nc.scalar.dma_start(out=bv_bc, in_=bvc[0:1, :].to_broadcast([128, c_dim]))
        ch = 512
        rr = [nc.sync, nc.gpsimd]
        j = 0
        # chunk 0 of y and x feeds the first K/Q projections and V tiles;
        # the remaining y chunks gate the attention tt-cursor and the JIT
        # V-projection, so they go before the remaining x chunks. The
        # scalar queue helps only with chunk 0 (done before exp starts).
        for et in range(n_et):
            cols = slice(0, ch)
            if et >= 6:
                nc.scalar.dma_start(out=yT_sb[:, et, cols], in_=yT[:, et, cols])
            else:
                rr[j % 2].dma_start(out=yT_sb[:, et, cols], in_=yT[:, et, cols])
                j += 1
        for et in range(n_et):
            cols = slice(0, ch)
            if et >= 6:
                nc.scalar.dma_start(out=xT_sb[:, et, cols], in_=xT[:, et, cols])
            else:
                rr[j % 2].dma_start(out=xT_sb[:, et, cols], in_=xT[:, et, cols])
                j += 1
        for c in range(1, t_len // ch):
            cols = slice(c * ch, (c + 1) * ch)
            for et in range(n_et):
                if c >= 2 and et < 2:
                    # a little of the later key stream on the scalar ring:
                    # few triggers, so the ACT FIFO never backs up on them
                    nc.scalar.dma_start(out=yT_sb[:, et, cols], in_=yT[:, et, cols])
                else:
                    rr[j % 2].dma_start(out=yT_sb[:, et, cols], in_=yT[:, et, cols])
                    j += 1
        for lo, hi in [(ch, 3 * ch), (3 * ch, s_loc)]:
            cols = slice(lo, hi)
            for et in range(n_et):
                rr[j % 2].dma_start(out=xT_sb[:, et, cols], in_=xT[:, et, cols])
                j += 1
        if n_hp > 1:
            nc.gpsimd.dma_start(out=wq_sb[:, 1:, :, :], in_=wq2[:, 1:, :, :])
            nc.sync.dma_start(out=wk_sb[:, 1:, :, :], in_=wk2[:, 1:, :, :])
        nc.gpsimd.dma_start(out=wo_sb, in_=wo)

        ones_row = consts.tile([1, 512], bf16)
        nc.vector.memset(ones_row, 1.0)
        # warm up the ACT exp table during the input DMA wait
        warm = consts.tile([1, 512], bf16)
        nc.scalar.activation(
            out=warm, in_=ones_row, func=_mybir().ActivationFunctionType.Exp
        )

        # V' with a ones column per head: [p, tt, head, 65]
        v_sb = consts.tile([128, n_tt, n_h, 65], bf16)
        nc.vector.memset(v_sb[:, :, :, 64:65], 1.0)
        oT_all = consts.tile([128, n_hp, s_loc], bf16)

        def v_proj_tt(tt):
            # V'[t-tile, all heads] in natural [t, c] layout; bias fused
            # into the DVE evacuation (bv broadcast tile)
            ps = acc_ps.tile([128, c_dim], f32, tag="acc")
            for et in range(n_et):
                nc.tensor.matmul(
                    out=ps,
                    lhsT=yT_sb[:, et, tt * 128 : (tt + 1) * 128],
                    rhs=wv_sb[:, et, :],
                    start=(et == 0),
                    stop=(et == n_et - 1),
                )
            nc.vector.tensor_add(
                v_sb[:, tt, :, 0:64],
                ps.rearrange("p (h d) -> p h d", d=64),
                bv_bc.rearrange("p (h d) -> p h d", d=64),
            )

        def k_chunk(hp, kt, c):
            ps = acc_ps.tile([128, tch], f32, tag="acc")
            for et in range(n_et):
                nc.tensor.matmul(
                    out=ps,
                    lhsT=wk_sb[:, hp, et, :],
                    rhs=yT_sb[:, et, c * tch : (c + 1) * tch],
                    start=(et == 0),
                    stop=(et == n_et - 1),
                )
            nc.vector.tensor_scalar_add(
                out=kt[:, c * tch : (c + 1) * tch],
                in0=ps,
                scalar1=bkc_sb[:, hp : hp + 1],
            )

        def q_chunk(hp, qt, c):
            ps = acc_ps.tile([128, qch], f32, tag="acc")
            for et in range(n_et):
                nc.tensor.matmul(
                    out=ps,
                    lhsT=wq_sb[:, hp, et, :],
                    rhs=xT_sb[:, et, c * qch : (c + 1) * qch],
                    start=(et == 0),
                    stop=(et == n_et - 1),
                )
            nc.vector.tensor_scalar_add(
                out=qt[:, c * qch : (c + 1) * qch],
                in0=ps,
                scalar1=bqc_sb[:, hp : hp + 1],
            )

        def qk_proj(hp):
            # K chunk first: it gates the first scores of qc0
            qt = qt_pool.tile([128, s_loc], bf16, tag="qt")
            kt = kt_pool.tile([128, t_len], bf16, tag="kt")
            for c in range(n_tc):
                k_chunk(hp, kt, c)
                q_chunk(hp, qt, c)
            return qt, kt

        def out_proj_unit(st, ec):
            # one partial-out tile: all heads' oT for this s-tile done
            ps = acc_ps.tile([128, ech], f32, tag="acc")
            for ct in range(n_hp):
                nc.tensor.matmul(
                    out=ps,
                    lhsT=oT_all[:, ct, st * 128 : (st + 1) * 128],
                    rhs=wo_sb[:, ct, ec * ech : (ec + 1) * ech],
                    start=(ct == 0),
                    stop=(ct == n_hp - 1),
                )
            o_sb = osb_pool.tile([128, ech], f32, tag="osb")
            nc.vector.tensor_copy(out=o_sb, in_=ps)
            nc.sync.dma_start(
                out=out[st * 128 : (st + 1) * 128, ec * ech : (ec + 1) * ech],
                in_=o_sb,
            )

        def out_proj_units(qc):
            return [
                (st, ec)
                for st in range(qc * (qch // 128), (qc + 1) * (qch // 128))
                for ec in range(n_ec)
            ]

        import math

        i32 = dt.int32
        a_schr = 0.125 * float(1 << 23) / math.log(2.0)
        b_schr = float((127 << 23) - 485000)

        def normalize(hp, qc, osc_a, osc_b, rcp_rows_dram):
            # partition-broadcast 1/sum via a DRAM round trip, then scale
            rbc_a = rbc_pool.tile([64, qch], bf16, tag="rbc")
            nc.sync.dma_start(
                out=rbc_a, in_=rcp_rows_dram[0:1, :].to_broadcast([64, qch])
            )
            rbc_b = rbc_pool.tile([64, qch], bf16, tag="rbc")
            nc.gpsimd.dma_start(
                out=rbc_b, in_=rcp_rows_dram[1:2, :].to_broadcast([64, qch])
            )
            nc.vector.tensor_mul(
                oT_all[0:64, hp, qc * qch : (qc + 1) * qch], osc_a, rbc_a
            )
            nc.vector.tensor_mul(
                oT_all[64:128, hp, qc * qch : (qc + 1) * qch], osc_b, rbc_b
            )

        pending_op = []  # out-proj units deferred into the next block
        for hp in range(n_hp):
            if hp == 0:
                # software-pipelined ramp: emit only what chunk-0 DMAs can
                # feed, then stream the rest just ahead of the tt cursor so
                # DMA-stalled matmuls never block the attention FIFO.
                qt = qt_pool.tile([128, s_loc], bf16, tag="qt")
                kt = kt_pool.tile([128, t_len], bf16, tag="kt")
                k_chunk(0, kt, 0)
                q_chunk(0, qt, 0)
                v_proj_tt(0)
                v_proj_tt(1)
            else:
                qt, kt = qk_proj(hp)
            last_hp = hp == n_hp - 1

            for qc in range(n_qc):
                o_a = o_ps_pool.tile([65, qch], f32, tag="o")
                o_b = o_ps_pool.tile([65, qch], f32, tag="o")
                for tt in range(n_tt):
                    sc_tile = sc_ps.tile([128, 2, qch], f32, tag="sc")
                    # scoresT for head a (contraction rows 0:64) and head b
                    # (rows 64:128) run concurrently via PE row tiling.
                    nc.tensor.matmul(
                        out=sc_tile[:, 0, :],
                        lhsT=kt[0:64, tt * 128 : (tt + 1) * 128],
                        rhs=qt[0:64, qc * qch : (qc + 1) * qch],
                        start=True,
                        stop=True,
                    )
                    nc.tensor.matmul(
                        out=sc_tile[:, 1, :],
                        lhsT=kt[64:128, tt * 128 : (tt + 1) * 128],
                        rhs=qt[64:128, qc * qch : (qc + 1) * qch],
                        start=True,
                        stop=True,
                    )
                    exp_t = exp_pool.tile([128, 2, qch], bf16, tag="exp")
                    if tt in dve_exp_tts:
                        # Schraudolph exp on DVE: bitcast(int(A*x + B))
                        ti = schr_pool.tile([128, 2, qch], i32, tag="ti")
                        nc.vector.tensor_scalar(
                            out=ti,
                            in0=sc_tile,
                            scalar1=a_schr,
                            scalar2=b_schr,
                            op0=_mybir().AluOpType.mult,
                            op1=_mybir().AluOpType.add,
                        )
                        nc.vector.tensor_copy(out=exp_t, in_=ti.bitcast(f32))
                    else:
                        nc.scalar.activation(
                            out=exp_t,
                            in_=sc_tile,
                            func=_mybir().ActivationFunctionType.Exp,
                            scale=0.125,
                        )
                    if debug_taps and hp == 0 and qc == 0 and tt == 0:
                        nc.sync.dma_start(out=dbg["exp0"], in_=exp_t)
                    first, last = tt == 0, tt == n_tt - 1
                    # attnV with ones column: row 64 = softmax denominator
                    nc.tensor.matmul(
                        out=o_a,
                        lhsT=v_sb[:, tt, 2 * hp, :],
                        rhs=exp_t[:, 0, :],
                        start=first,
                        stop=last,
                    )
                    nc.tensor.matmul(
                        out=o_b,
                        lhsT=v_sb[:, tt, 2 * hp + 1, :],
                        rhs=exp_t[:, 1, :],
                        start=first,
                        stop=last,
                    )
                    if hp == 0 and qc == 0:
                        # stream the remaining V tiles and K/Q chunks just
                        # ahead of where the attention will need them
                        if tt + 2 < n_tt:
                            v_proj_tt(tt + 2)
                        if tt in (1, 5, 9):
                            k_chunk(0, kt, 1 + (tt - 1) // 4)
                        if tt in (3, 7, 11):
                            q_chunk(0, qt, 1 + (tt - 3) // 4)
                    elif pending_op and (tt % 2 == 0 and tt >= 6 or tt == 15):
                        # spread the previous chunk's out-proj tiles thinly
                        # so their psum-evac latency never stalls the PE FIFO
                        out_proj_unit(*pending_op.pop(0))
                # psum evacuation: denominators first (they head the
                # reciprocal/broadcast latency chain), then the data rows
                den_a = rcp_pool.tile([1, qch], f32, tag="den")
                nc.vector.tensor_copy(out=den_a, in_=o_a[64:65, :])
                den_b = rcp_pool.tile([1, qch], f32, tag="den")
                nc.vector.tensor_copy(out=den_b, in_=o_b[64:65, :])
                osc_a = osc_pool.tile([64, qch], bf16, tag="osc")
                nc.vector.tensor_copy(out=osc_a, in_=o_a[0:64, :])
                osc_b = osc_pool.tile([64, qch], bf16, tag="osc")
                nc.vector.tensor_copy(out=osc_b, in_=o_b[0:64, :])
                if debug_taps and hp == 0 and qc == 0:
                    nc.sync.dma_start(out=dbg["osc0"], in_=osc_a)
                    nc.sync.dma_start(out=dbg["den0"], in_=den_a)
                # gather both denominators on adjacent partitions, one
                # reciprocal call for the pair (cost is free-dim bound)
                den2 = rcp_pool.tile([2, qch], f32, tag="den2")
                nc.sync.dma_start(out=den2[0:1, :], in_=den_a)
                nc.gpsimd.dma_start(out=den2[1:2, :], in_=den_b)
                nc.vector.reciprocal(out=den2, in_=den2)
                rcp2 = rcp_pool.tile([2, qch], bf16, tag="rcp2")
                nc.vector.tensor_copy(out=rcp2, in_=den2)
                if debug_taps and hp == 0 and qc == 0:
                    nc.sync.dma_start(out=dbg["rcp0"], in_=den2[0:1, :])
                scr = scr_pool.tile([2, qch], bf16, tag="scr")
                nc.sync.dma_start(out=scr, in_=rcp2)
                normalize(hp, qc, osc_a, osc_b, scr)
                if last_hp:
                    pending_op.extend(out_proj_units(qc))
                    if qc == n_qc - 1:
                        while pending_op:
                            out_proj_unit(*pending_op.pop(0))

            if debug_taps and hp == 0:
                nc.sync.dma_start(out=dbg["qt0"], in_=qt)
                nc.sync.dma_start(out=dbg["kt0"], in_=kt)

        if debug_taps:
            nc.sync.dma_start(out=dbg["v"], in_=v_sb)
            nc.sync.dma_start(out=dbg["oT"], in_=oT_all)

    nc.compile()
    return nc


def _bf16(a):
    import ml_dtypes

    return np.ascontiguousarray(a).astype(ml_dtypes.bfloat16)


def host_prep_half(Wq, bq, Wk, bk, Wv, bv, Wo, h0, n_hp=4, n_et=8):
    """Pack one head-half's weights into the kernel's DRAM layouts."""
    e_dim = 128 * n_et
    n_heads = 2 * n_hp

    def pack_pairs(W):
        # [H, E, D] -> [p, hp, et, m] with m = j*64+d, head = h0 + 2*hp+j
        Wr = W[h0 : h0 + n_heads].reshape(n_hp, 2, e_dim, D)  # hp, j, e, d
        arr = Wr.transpose(2, 0, 1, 3).reshape(e_dim, n_hp, 128)  # e, hp, m
        arr = arr.reshape(n_et, 128, n_hp, 128).transpose(1, 2, 0, 3)
        return np.ascontiguousarray(arr)  # [p, hp, et, m]

    def bias_cols(b):
        # [H, D] -> [p, hp] with p = j*64+d
        return np.ascontiguousarray(
            b[h0 : h0 + n_heads]
            .reshape(n_hp, 2, 64)
            .transpose(1, 2, 0)
            .reshape(128, n_hp)
        ).astype(np.float32)

    c_dim = 64 * n_heads
    wv_cat = Wv[h0 : h0 + n_heads].transpose(1, 0, 2).reshape(e_dim, c_dim)  # [e, c]
    wv_arr = wv_cat.reshape(n_et, 128, c_dim).transpose(1, 0, 2)  # [p, et, c]
    wo_arr = (
        Wo[h0 * D : (h0 + n_heads) * D].reshape(n_hp, 128, e_dim).transpose(1, 0, 2)
    )  # [p, ct, e]

    return {
        "wq2": _bf16(pack_pairs(Wq)),
        "wk2": _bf16(pack_pairs(Wk)),
        "wv": _bf16(np.ascontiguousarray(wv_arr)),
        "wo": _bf16(np.ascontiguousarray(wo_arr)),
        "bqc": bias_cols(bq),
        "bkc": bias_cols(bk),
        "bvc": _bf16(bv[h0 : h0 + n_heads].reshape(1, c_dim)),
    }


def host_prep_xt(mat, n_et=8):
    """[rows, E] -> [p, et, rows] transposed tiled layout, bf16."""
    rows, e_dim = mat.shape
    assert e_dim == 128 * n_et
    arr = mat.T.reshape(n_et, 128, rows).transpose(1, 0, 2)
    return _bf16(arr)


def kernel(x, y, Wq, bq, Wk, bk, Wv, bv, Wo, bo):
    import os
    import sys

    if "/opt/trn_rl_repo" not in sys.path:
        sys.path.insert(0, "/opt/trn_rl_repo")
    from concourse import bass_utils

    x = np.asarray(x, dtype=np.float32)
    y = np.asarray(y, dtype=np.float32)

    if "prog" not in _compiled:
        _compiled["prog"] = build_program()
    nc = _compiled["prog"]

    Wq, bq = np.asarray(Wq, np.float32), np.asarray(bq, np.float32)
    Wk, bk = np.asarray(Wk, np.float32), np.asarray(bk, np.float32)
    Wv, bv = np.asarray(Wv, np.float32), np.asarray(bv, np.float32)
    Wo, bo = np.asarray(Wo, np.float32), np.asarray(bo, np.float32)

    halves = [host_prep_half(Wq, bq, Wk, bk, Wv, bv, Wo, hh * 8) for hh in range(2)]
    xT_b = [host_prep_xt(x[b]) for b in range(B)]
    yT_b = [host_prep_xt(y[b]) for b in range(B)]
    in_maps = []
    for c in range(N_CORES):
        b, hh = c // 2, c % 2
        m = dict(halves[hh])
        m["xT"] = xT_b[b]
        m["yT"] = yT_b[b]
        in_maps.append(m)

    trace = os.environ.get("TRN_ATTN_TRACE", "0") == "1"
    res = bass_utils.run_bass_kernel_spmd(
        nc, in_maps, core_ids=list(range(N_CORES)), trace=trace
    )
    _compiled["last_results"] = res
    out = np.empty((B, S, E), dtype=np.float32)
    for b in range(B):
        out[b] = res.results[2 * b]["out"]
        out[b] += res.results[2 * b + 1]["out"]
        out[b] += bo
    return out


# revision 40
# speedup vs baseline: 1.1531x; 1.1531x over previous
"""Cross multi-head attention on 8 Trainium2 NeuronCores.

Sharding: tensor-parallel over heads x data-parallel over batch.
Core c handles batch b = c//2 and head-half hh = c%2 (8 of 16 heads),
full S=2048 queries, full T=2048 keys. No K/V recompute (the old
batch x seq-half layout projected K/V twice per batch). Each core
emits a PARTIAL output (its 8 heads' concat @ Wo slice, no bias);
the host sums the two partials per batch and adds bo. That host-side
reduce is the tensor-parallel unshard for the contraction split of
combo_linear.

Per-core kernel (T-major layout, no on-chip transposes; host
pre-transposes x/y and pre-packs weights):
  QT[hp]  [128,S]  = Wq2[hp].T @ xT      (head-pair packed: rows 0:64 head a,
  KT[hp]  [128,T]  = Wk2[hp].T @ yT       rows 64:128 head b; bias fused into
                                          the PSUM->SBUF copy on DVE)
  V'      [t,h,65] = yT.T @ Wv_cat | 1   (natural layout + ones column;
                                          projected per t-tile just-in-time
                                          inside the first attention pass)
  scoresT [t,q]    = KT_h.T @ QT_h       (K=64 contraction; two heads truly
                                          concurrent via PE row tiling)
  expT    = exp(scoresT * 0.125)         (one ACT op per psum pair tile)
  oT'     += V'_h.T @ expT               (M=65: row 64 accumulates the softmax
                                          denominator for free)
  oT      = oT'[0:64] * bcast(1/oT'[64]) (deferred normalization; fast
                                          approximate reciprocal on DVE)
  partial = concatT.T @ Wo               (per-qc, interleaved into the last
                                          head-pair's attention)
Matmul inputs bf16 (fp32 PSUM accumulation), softmax in fp32.
"""

import numpy as np

B, S, T, E, H, D = 4, 2048, 2048, 1024, 16, 64
N_CORES = 8

_compiled = {}


def _dt():
    from concourse import mybir

    return mybir.dt


def _mybir():
    from concourse import mybir

    return mybir


def build_program(n_hp=4, s_loc=2048, t_len=2048, n_et=8, dve_exp_tts=(), debug_taps=False):
    """Emit the per-core bass program.

    n_hp: head pairs on this core (heads = 2*n_hp), s_loc: query rows,
    t_len: key rows, n_et: contraction tiles (emb dim = 128*n_et).
    dve_exp_tts: set of tt indices whose softmax exp runs on DVE via the
    Schraudolph bit trick instead of ACT (load balancing knob).
    """
    import concourse.tile as tile
    from concourse import bacc

    dt = _dt()
    bf16 = dt.bfloat16
    f32 = dt.float32

    e_dim = 128 * n_et
    c_dim = 64 * 2 * n_hp  # concat dim of this core's heads
    n_h = 2 * n_hp
    n_tt = t_len // 128  # key tiles
    qch = min(512, s_loc)  # query chunk width
    tch = min(512, t_len)
    ech = min(512, e_dim)
    n_qc = s_loc // qch  # query chunks for attention
    n_st = s_loc // 128  # output row tiles
    n_ec = e_dim // ech  # output col chunks
    n_tc = t_len // tch

    nc = bacc.Bacc("TRN2", target_bir_lowering=False, debug=False)

    # ---- DRAM I/O (host provides these layouts directly) ----
    xT = nc.dram_tensor("xT", [128, n_et, s_loc], bf16, kind="ExternalInput").ap()
    yT = nc.dram_tensor("yT", [128, n_et, t_len], bf16, kind="ExternalInput").ap()
    wq2 = nc.dram_tensor("wq2", [128, n_hp, n_et, 128], bf16, kind="ExternalInput").ap()
    wk2 = nc.dram_tensor("wk2", [128, n_hp, n_et, 128], bf16, kind="ExternalInput").ap()
    wv = nc.dram_tensor("wv", [128, n_et, c_dim], bf16, kind="ExternalInput").ap()
    wo = nc.dram_tensor("wo", [128, n_hp, e_dim], bf16, kind="ExternalInput").ap()
    bqc = nc.dram_tensor("bqc", [128, n_hp], f32, kind="ExternalInput").ap()
    bkc = nc.dram_tensor("bkc", [128, n_hp], f32, kind="ExternalInput").ap()
    bvc = nc.dram_tensor("bvc", [1, c_dim], bf16, kind="ExternalInput").ap()
    out = nc.dram_tensor("out", [s_loc, e_dim], f32, kind="ExternalOutput").ap()

    from contextlib import ExitStack

    dbg = {}
    if debug_taps:
        dbg["qt0"] = nc.dram_tensor("dbg_qt0", [128, s_loc], dt.bfloat16, kind="ExternalOutput").ap()
        dbg["kt0"] = nc.dram_tensor("dbg_kt0", [128, t_len], dt.bfloat16, kind="ExternalOutput").ap()
        dbg["v"] = nc.dram_tensor("dbg_v", [128, t_len // 128, 2 * n_hp, 65], dt.bfloat16, kind="ExternalOutput").ap()
        dbg["oT"] = nc.dram_tensor("dbg_oT", [128, n_hp, s_loc], dt.bfloat16, kind="ExternalOutput").ap()
        dbg["osc0"] = nc.dram_tensor("dbg_osc0", [64, min(512, s_loc)], dt.bfloat16, kind="ExternalOutput").ap()
        dbg["den0"] = nc.dram_tensor("dbg_den0", [1, min(512, s_loc)], f32, kind="ExternalOutput").ap()
        dbg["rcp0"] = nc.dram_tensor("dbg_rcp0", [1, min(512, s_loc)], f32, kind="ExternalOutput").ap()
        dbg["exp0"] = nc.dram_tensor("dbg_exp0", [128, 2, min(512, s_loc)], dt.bfloat16, kind="ExternalOutput").ap()

    with tile.TileContext(nc) as tc, ExitStack() as ctx:
        consts = ctx.enter_context(tc.tile_pool(name="consts", bufs=1))
        scr_pool = ctx.enter_context(tc.tile_pool(name="scr", bufs=2, space="DRAM"))
        qt_pool = ctx.enter_context(tc.tile_pool(name="qt", bufs=2))
        kt_pool = ctx.enter_context(tc.tile_pool(name="kt", bufs=2))
        exp_pool = ctx.enter_context(tc.tile_pool(name="expp", bufs=5))
        schr_pool = ctx.enter_context(tc.tile_pool(name="schr", bufs=1))
        osc_pool = ctx.enter_context(tc.tile_pool(name="osc", bufs=8))
        rcp_pool = ctx.enter_context(tc.tile_pool(name="rcp", bufs=4))
        den_pool = ctx.enter_context(tc.tile_pool(name="den", bufs=2))
        rbc_pool = ctx.enter_context(tc.tile_pool(name="rbc", bufs=4))
        osb_pool = ctx.enter_context(tc.tile_pool(name="osb", bufs=2))
        sc_ps = ctx.enter_context(tc.tile_pool(name="scps", bufs=2, space="PSUM"))
        acc_ps = ctx.enter_context(tc.tile_pool(name="accps", bufs=2, space="PSUM"))
        o_ps_pool = ctx.enter_context(tc.tile_pool(name="ops", bufs=2, space="PSUM"))

        # ---- resident tiles ----
        xT_sb = consts.tile([128, n_et, s_loc], bf16)
        yT_sb = consts.tile([128, n_et, t_len], bf16)
        wq_sb = consts.tile([128, n_hp, n_et, 128], bf16)
        wk_sb = consts.tile([128, n_hp, n_et, 128], bf16)
        wv_sb = consts.tile([128, n_et, c_dim], bf16)
        wo_sb = consts.tile([128, n_hp, e_dim], bf16)
        bqc_sb = consts.tile([128, n_hp], f32)
        bkc_sb = consts.tile([128, n_hp], f32)
        bv_sb = consts.tile([1, c_dim], bf16)

        # DMA ordering: the critical path to the first attention tile is
        # yT chunk0 -> K-proj hp0 chunk0 -> scores (plus wv for the JIT
        # V-projection). 512-col chunks keep region deps fine-grained so
        # compute starts as soon as the first column chunk of each et row
        # lands; spread across the three DMA-capable engine queues.
        bv_bc = consts.tile([128, c_dim], bf16)
        nc.sync.dma_start(out=wk_sb[:, 0, :, :], in_=wk2[:, 0, :, :])
        nc.gpsimd.dma_start(out=wq_sb[:, 0, :, :], in_=wq2[:, 0, :, :])
        # the scalar (ACT) queue gets ONLY early small transfers: every
        # dma_start on it occupies the ACT instruction FIFO, so bulk loads
        # there would stall the exp pipeline behind the DMA ring
        nc.scalar.dma_start(out=bqc_sb, in_=bqc)
        nc.scalar.dma_start(out=bkc_sb, in_=bkc)
        nc.scalar.dma_start(out=bv_sb, in_=bvc)
        # wv gates the JIT V-projection right behind the first K/Q chunks
        nc.sync.dma_start(out=wv_sb[:, :, 0 : c_dim // 2], in_=wv[:, :, 0 : c_dim // 2])
        nc.gpsimd.dma_start(out=wv_sb[:, :, c_dim // 2 :], in_=wv[:, :, c_dim // 2 :])
        nc.scalar.dma_start(out=bv_bc, in_=bvc[0:1, :].to_broadcast([128, c_dim]))
        ch = 512
        rr = [nc.sync, nc.gpsimd]
        j = 0
        # chunk 0 of y and x feeds the first K/Q projections and V tiles;
        # the remaining y chunks gate the attention tt-cursor and the JIT
        # V-projection, so they go before the remaining x chunks. The
        # scalar queue helps only with chunk 0 (done before exp starts).
        for et in range(n_et):
            cols = slice(0, ch)
            if et >= 6:
                nc.scalar.dma_start(out=yT_sb[:, et, cols], in_=yT[:, et, cols])
            else:
                rr[j % 2].dma_start(out=yT_sb[:, et, cols], in_=yT[:, et, cols])
                j += 1
        for et in range(n_et):
            cols = slice(0, ch)
            if et >= 6:
                nc.scalar.dma_start(out=xT_sb[:, et, cols], in_=xT[:, et, cols])
            else:
                rr[j % 2].dma_start(out=xT_sb[:, et, cols], in_=xT[:, et, cols])
                j += 1
        for c in range(1, t_len // ch):
            cols = slice(c * ch, (c + 1) * ch)
            for et in range(n_et):
                rr[j % 2].dma_start(out=yT_sb[:, et, cols], in_=yT[:, et, cols])
                j += 1
        for lo, hi in [(ch, 3 * ch), (3 * ch, s_loc)]:
            cols = slice(lo, hi)
            for et in range(n_et):
                rr[j % 2].dma_start(out=xT_sb[:, et, cols], in_=xT[:, et, cols])
                j += 1
        if n_hp > 1:
            nc.gpsimd.dma_start(out=wq_sb[:, 1:, :, :], in_=wq2[:, 1:, :, :])
            nc.sync.dma_start(out=wk_sb[:, 1:, :, :], in_=wk2[:, 1:, :, :])
        nc.gpsimd.dma_start(out=wo_sb, in_=wo)

        ones_row = consts.tile([1, 512], bf16)
        nc.vector.memset(ones_row, 1.0)
        # warm up the ACT exp table during the input DMA wait
        warm = consts.tile([1, 512], bf16)
        nc.scalar.activation(
            out=warm, in_=ones_row, func=_mybir().ActivationFunctionType.Exp
        )

        # V' with a ones column per head: [p, tt, head, 65]
        v_sb = consts.tile([128, n_tt, n_h, 65], bf16)
        nc.vector.memset(v_sb[:, :, :, 64:65], 1.0)
        oT_all = consts.tile([128, n_hp, s_loc], bf16)

        def v_proj_tt(tt):
            # V'[t-tile, all heads] in natural [t, c] layout; bias fused
            # into the DVE evacuation (bv broadcast tile)
            ps = acc_ps.tile([128, c_dim], f32, tag="acc")
            for et in range(n_et):
                nc.tensor.matmul(
                    out=ps,
                    lhsT=yT_sb[:, et, tt * 128 : (tt + 1) * 128],
                    rhs=wv_sb[:, et, :],
                    start=(et == 0),
                    stop=(et == n_et - 1),
                )
            nc.vector.tensor_add(
                v_sb[:, tt, :, 0:64],
                ps.rearrange("p (h d) -> p h d", d=64),
                bv_bc.rearrange("p (h d) -> p h d", d=64),
            )

        def k_chunk(hp, kt, c):
            ps = acc_ps.tile([128, tch], f32, tag="acc")
            for et in range(n_et):
                nc.tensor.matmul(
                    out=ps,
                    lhsT=wk_sb[:, hp, et, :],
                    rhs=yT_sb[:, et, c * tch : (c + 1) * tch],
                    start=(et == 0),
                    stop=(et == n_et - 1),
                )
            nc.vector.tensor_scalar_add(
                out=kt[:, c * tch : (c + 1) * tch],
                in0=ps,
                scalar1=bkc_sb[:, hp : hp + 1],
            )

        def q_chunk(hp, qt, c):
            ps = acc_ps.tile([128, qch], f32, tag="acc")
            for et in range(n_et):
                nc.tensor.matmul(
                    out=ps,
                    lhsT=wq_sb[:, hp, et, :],
                    rhs=xT_sb[:, et, c * qch : (c + 1) * qch],
                    start=(et == 0),
                    stop=(et == n_et - 1),
                )
            nc.vector.tensor_scalar_add(
                out=qt[:, c * qch : (c + 1) * qch],
                in0=ps,
                scalar1=bqc_sb[:, hp : hp + 1],
            )

        def qk_proj(hp):
            # K chunk first: it gates the first scores of qc0
            qt = qt_pool.tile([128, s_loc], bf16, tag="qt")
            kt = kt_pool.tile([128, t_len], bf16, tag="kt")
            for c in range(n_tc):
                k_chunk(hp, kt, c)
                q_chunk(hp, qt, c)
            return qt, kt

        def out_proj_unit(st, ec):
            # one partial-out tile: all heads' oT for this s-tile done
            ps = acc_ps.tile([128, ech], f32, tag="acc")
            for ct in range(n_hp):
                nc.tensor.matmul(
                    out=ps,
                    lhsT=oT_all[:, ct, st * 128 : (st + 1) * 128],
                    rhs=wo_sb[:, ct, ec * ech : (ec + 1) * ech],
                    start=(ct == 0),
                    stop=(ct == n_hp - 1),
                )
            o_sb = osb_pool.tile([128, ech], f32, tag="osb")
            nc.vector.tensor_copy(out=o_sb, in_=ps)
            nc.sync.dma_start(
                out=out[st * 128 : (st + 1) * 128, ec * ech : (ec + 1) * ech],
                in_=o_sb,
            )

        def out_proj_units(qc):
            return [
                (st, ec)
                for st in range(qc * (qch // 128), (qc + 1) * (qch // 128))
                for ec in range(n_ec)
            ]

        import math

        i32 = dt.int32
        a_schr = 0.125 * float(1 << 23) / math.log(2.0)
        b_schr = float((127 << 23) - 485000)

        def normalize(hp, qc, osc_a, osc_b, rcp_rows_dram):
            # partition-broadcast 1/sum via a DRAM round trip, then scale
            rbc_a = rbc_pool.tile([64, qch], bf16, tag="rbc")
            nc.sync.dma_start(
                out=rbc_a, in_=rcp_rows_dram[0:1, :].to_broadcast([64, qch])
            )
            rbc_b = rbc_pool.tile([64, qch], bf16, tag="rbc")
            nc.gpsimd.dma_start(
                out=rbc_b, in_=rcp_rows_dram[1:2, :].to_broadcast([64, qch])
            )
            nc.vector.tensor_mul(
                oT_all[0:64, hp, qc * qch : (qc + 1) * qch], osc_a, rbc_a
            )
            nc.vector.tensor_mul(
                oT_all[64:128, hp, qc * qch : (qc + 1) * qch], osc_b, rbc_b
            )

        pending_op = []  # out-proj units deferred into the next block
        for hp in range(n_hp):
            if hp == 0:
                # software-pipelined ramp: emit only what chunk-0 DMAs can
                # feed, then stream the rest just ahead of the tt cursor so
                # DMA-stalled matmuls never block the attention FIFO.
                qt = qt_pool.tile([128, s_loc], bf16, tag="qt")
                kt = kt_pool.tile([128, t_len], bf16, tag="kt")
                k_chunk(0, kt, 0)
                q_chunk(0, qt, 0)
                v_proj_tt(0)
                v_proj_tt(1)
            else:
                qt, kt = qk_proj(hp)
            last_hp = hp == n_hp - 1

            for qc in range(n_qc):
                o_a = o_ps_pool.tile([65, qch], f32, tag="o")
                o_b = o_ps_pool.tile([65, qch], f32, tag="o")
                for tt in range(n_tt):
                    sc_tile = sc_ps.tile([128, 2, qch], f32, tag="sc")
                    # scoresT for head a (contraction rows 0:64) and head b
                    # (rows 64:128) run concurrently via PE row tiling.
                    nc.tensor.matmul(
                        out=sc_tile[:, 0, :],
                        lhsT=kt[0:64, tt * 128 : (tt + 1) * 128],
                        rhs=qt[0:64, qc * qch : (qc + 1) * qch],
                        start=True,
                        stop=True,
                    )
                    nc.tensor.matmul(
                        out=sc_tile[:, 1, :],
                        lhsT=kt[64:128, tt * 128 : (tt + 1) * 128],
                        rhs=qt[64:128, qc * qch : (qc + 1) * qch],
                        start=True,
                        stop=True,
                    )
                    exp_t = exp_pool.tile([128, 2, qch], bf16, tag="exp")
                    if tt in dve_exp_tts:
                        # Schraudolph exp on DVE: bitcast(int(A*x + B))
                        ti = schr_pool.tile([128, 2, qch], i32, tag="ti")
                        nc.vector.tensor_scalar(
                            out=ti,
                            in0=sc_tile,
                            scalar1=a_schr,
                            scalar2=b_schr,
                            op0=_mybir().AluOpType.mult,
                            op1=_mybir().AluOpType.add,
                        )
                        nc.vector.tensor_copy(out=exp_t, in_=ti.bitcast(f32))
                    else:
                        nc.scalar.activation(
                            out=exp_t,
                            in_=sc_tile,
                            func=_mybir().ActivationFunctionType.Exp,
                            scale=0.125,
                        )
                    if debug_taps and hp == 0 and qc == 0 and tt == 0:
                        nc.sync.dma_start(out=dbg["exp0"], in_=exp_t)
                    first, last = tt == 0, tt == n_tt - 1
                    # attnV with ones column: row 64 = softmax denominator
                    nc.tensor.matmul(
                        out=o_a,
                        lhsT=v_sb[:, tt, 2 * hp, :],
                        rhs=exp_t[:, 0, :],
                        start=first,
                        stop=last,
                    )
                    nc.tensor.matmul(
                        out=o_b,
                        lhsT=v_sb[:, tt, 2 * hp + 1, :],
                        rhs=exp_t[:, 1, :],
                        start=first,
                        stop=last,
                    )
                    if hp == 0 and qc == 0:
                        # stream the remaining V tiles and K/Q chunks just
                        # ahead of where the attention will need them
                        if tt + 2 < n_tt:
                            v_proj_tt(tt + 2)
                        if tt in (1, 5, 9):
                            k_chunk(0, kt, 1 + (tt - 1) // 4)
                        if tt in (3, 7, 11):
                            q_chunk(0, qt, 1 + (tt - 3) // 4)
                    elif pending_op and tt % 2 == 0 and tt >= 2:
                        # spread the previous chunk's out-proj tiles thinly
                        # so their psum-evac latency never stalls the PE FIFO
                        out_proj_unit(*pending_op.pop(0))
                # psum evacuation: data rows to bf16, denominator rows to f32
                osc_a = osc_pool.tile([64, qch], bf16, tag="osc")
                nc.vector.tensor_copy(out=osc_a, in_=o_a[0:64, :])
                osc_b = osc_pool.tile([64, qch], bf16, tag="osc")
                nc.vector.tensor_copy(out=osc_b, in_=o_b[0:64, :])
                den_a = rcp_pool.tile([1, qch], f32, tag="den")
                nc.vector.tensor_copy(out=den_a, in_=o_a[64:65, :])
                den_b = rcp_pool.tile([1, qch], f32, tag="den")
                nc.vector.tensor_copy(out=den_b, in_=o_b[64:65, :])
                if debug_taps and hp == 0 and qc == 0:
                    nc.sync.dma_start(out=dbg["osc0"], in_=osc_a)
                    nc.sync.dma_start(out=dbg["den0"], in_=den_a)
                # gather both denominators on adjacent partitions, one
                # reciprocal call for the pair (cost is free-dim bound)
                den2 = rcp_pool.tile([2, qch], f32, tag="den2")
                nc.sync.dma_start(out=den2[0:1, :], in_=den_a)
                nc.gpsimd.dma_start(out=den2[1:2, :], in_=den_b)
                nc.vector.reciprocal(out=den2, in_=den2)
                rcp2 = rcp_pool.tile([2, qch], bf16, tag="rcp2")
                nc.vector.tensor_copy(out=rcp2, in_=den2)
                if debug_taps and hp == 0 and qc == 0:
                    nc.sync.dma_start(out=dbg["rcp0"], in_=den2[0:1, :])
                scr = scr_pool.tile([2, qch], bf16, tag="scr")
                nc.sync.dma_start(out=scr, in_=rcp2)
                normalize(hp, qc, osc_a, osc_b, scr)
                if last_hp:
                    pending_op.extend(out_proj_units(qc))
                    if qc == n_qc - 1:
                        while pending_op:
                            out_proj_unit(*pending_op.pop(0))

            if debug_taps and hp == 0:
                nc.sync.dma_start(out=dbg["qt0"], in_=qt)
                nc.sync.dma_start(out=dbg["kt0"], in_=kt)

        if debug_taps:
            nc.sync.dma_start(out=dbg["v"], in_=v_sb)
            nc.sync.dma_start(out=dbg["oT"], in_=oT_all)

    nc.compile()
    return nc


def _bf16(a):
    import ml_dtypes

    return np.ascontiguousarray(a).astype(ml_dtypes.bfloat16)


def host_prep_half(Wq, bq, Wk, bk, Wv, bv, Wo, h0, n_hp=4, n_et=8):
    """Pack one head-half's weights into the kernel's DRAM layouts."""
    e_dim = 128 * n_et
    n_heads = 2 * n_hp

    def pack_pairs(W):
        # [H, E, D] -> [p, hp, et, m] with m = j*64+d, head = h0 + 2*hp+j
        Wr = W[h0 : h0 + n_heads].reshape(n_hp, 2, e_dim, D)  # hp, j, e, d
        arr = Wr.transpose(2, 0, 1, 3).reshape(e_dim, n_hp, 128)  # e, hp, m
        arr = arr.reshape(n_et, 128, n_hp, 128).transpose(1, 2, 0, 3)
        return np.ascontiguousarray(arr)  # [p, hp, et, m]

    def bias_cols(b):
        # [H, D] -> [p, hp] with p = j*64+d
        return np.ascontiguousarray(
            b[h0 : h0 + n_heads]
            .reshape(n_hp, 2, 64)
            .transpose(1, 2, 0)
            .reshape(128, n_hp)
        ).astype(np.float32)

    c_dim = 64 * n_heads
    wv_cat = Wv[h0 : h0 + n_heads].transpose(1, 0, 2).reshape(e_dim, c_dim)  # [e, c]
    wv_arr = wv_cat.reshape(n_et, 128, c_dim).transpose(1, 0, 2)  # [p, et, c]
    wo_arr = (
        Wo[h0 * D : (h0 + n_heads) * D].reshape(n_hp, 128, e_dim).transpose(1, 0, 2)
    )  # [p, ct, e]

    return {
        "wq2": _bf16(pack_pairs(Wq)),
        "wk2": _bf16(pack_pairs(Wk)),
        "wv": _bf16(np.ascontiguousarray(wv_arr)),
        "wo": _bf16(np.ascontiguousarray(wo_arr)),
        "bqc": bias_cols(bq),
        "bkc": bias_cols(bk),
        "bvc": _bf16(bv[h0 : h0 + n_heads].reshape(1, c_dim)),
    }


def host_prep_xt(mat, n_et=8):
    """[rows, E] -> [p, et, rows] transposed tiled layout, bf16."""
    rows, e_dim = mat.shape
    assert e_dim == 128 * n_et
    arr = mat.T.reshape(n_et, 128, rows).transpose(1, 0, 2)
    return _bf16(arr)


def kernel(x, y, Wq, bq, Wk, bk, Wv, bv, Wo, bo):
    import os
    import sys

    if "/opt/trn_rl_repo" not in sys.path:
        sys.path.insert(0, "/opt/trn_rl_repo")
    from concourse import bass_utils

    x = np.asarray(x, dtype=np.float32)
    y = np.asarray(y, dtype=np.float32)

    if "prog" not in _compiled:
        _compiled["prog"] = build_program()
    nc = _compiled["prog"]

    Wq, bq = np.asarray(Wq, np.float32), np.asarray(bq, np.float32)
    Wk, bk = np.asarray(Wk, np.float32), np.asarray(bk, np.float32)
    Wv, bv = np.asarray(Wv, np.float32), np.asarray(bv, np.float32)
    Wo, bo = np.asarray(Wo, np.float32), np.asarray(bo, np.float32)

    halves = [host_prep_half(Wq, bq, Wk, bk, Wv, bv, Wo, hh * 8) for hh in range(2)]
    xT_b = [host_prep_xt(x[b]) for b in range(B)]
    yT_b = [host_prep_xt(y[b]) for b in range(B)]
    in_maps = []
    for c in range(N_CORES):
        b, hh = c // 2, c % 2
        m = dict(halves[hh])
        m["xT"] = xT_b[b]
        m["yT"] = yT_b[b]
        in_maps.append(m)

    trace = os.environ.get("TRN_ATTN_TRACE", "0") == "1"
    res = bass_utils.run_bass_kernel_spmd(
        nc, in_maps, core_ids=list(range(N_CORES)), trace=trace
    )
    _compiled["last_results"] = res
    out = np.empty((B, S, E), dtype=np.float32)
    for b in range(B):
        out[b] = res.results[2 * b]["out"]
        out[b] += res.results[2 * b + 1]["out"]
        out[b] += bo
    return out


# revision 41
# speedup vs baseline: 1.1660x; 1.0112x over previous
"""Cross multi-head attention on 8 Trainium2 NeuronCores.

Sharding: tensor-parallel over heads x data-parallel over batch.
Core c handles batch b = c//2 and head-half hh = c%2 (8 of 16 heads),
full S=2048 queries, full T=2048 keys. No K/V recompute (the old
batch x seq-half layout projected K/V twice per batch). Each core
emits a PARTIAL output (its 8 heads' concat @ Wo slice, no bias);
the host sums the two partials per batch and adds bo. That host-side
reduce is the tensor-parallel unshard for the contraction split of
combo_linear.

Per-core kernel (T-major layout, no on-chip transposes; host
pre-transposes x/y and pre-packs weights):
  QT[hp]  [128,S]  = Wq2[hp].T @ xT      (head-pair packed: rows 0:64 head a,
  KT[hp]  [128,T]  = Wk2[hp].T @ yT       rows 64:128 head b; bias fused into
                                          the PSUM->SBUF copy on DVE)
  V'      [t,h,65] = yT.T @ Wv_cat | 1   (natural layout + ones column;
                                          projected per t-tile just-in-time
                                          inside the first attention pass)
  scoresT [t,q]    = KT_h.T @ QT_h       (K=64 contraction; two heads truly
                                          concurrent via PE row tiling)
  expT    = exp(scoresT * 0.125)         (one ACT op per psum pair tile)
  oT'     += V'_h.T @ expT               (M=65: row 64 accumulates the softmax
                                          denominator for free)
  oT      = oT'[0:64] * bcast(1/oT'[64]) (deferred normalization; fast
                                          approximate reciprocal on DVE)
  partial = concatT.T @ Wo               (per-qc, interleaved into the last
                                          head-pair's attention)
Matmul inputs bf16 (fp32 PSUM accumulation), softmax in fp32.
"""

import numpy as np

B, S, T, E, H, D = 4, 2048, 2048, 1024, 16, 64
N_CORES = 8

_compiled = {}


def _dt():
    from concourse import mybir

    return mybir.dt


def _mybir():
    from concourse import mybir

    return mybir


def build_program(n_hp=4, s_loc=2048, t_len=2048, n_et=8, dve_exp_tts=(), debug_taps=False):
    """Emit the per-core bass program.

    n_hp: head pairs on this core (heads = 2*n_hp), s_loc: query rows,
    t_len: key rows, n_et: contraction tiles (emb dim = 128*n_et).
    dve_exp_tts: set of tt indices whose softmax exp runs on DVE via the
    Schraudolph bit trick instead of ACT (load balancing knob).
    """
    import concourse.tile as tile
    from concourse import bacc

    dt = _dt()
    bf16 = dt.bfloat16
    f32 = dt.float32

    e_dim = 128 * n_et
    c_dim = 64 * 2 * n_hp  # concat dim of this core's heads
    n_h = 2 * n_hp
    n_tt = t_len // 128  # key tiles
    qch = min(512, s_loc)  # query chunk width
    tch = min(512, t_len)
    ech = min(512, e_dim)
    n_qc = s_loc // qch  # query chunks for attention
    n_st = s_loc // 128  # output row tiles
    n_ec = e_dim // ech  # output col chunks
    n_tc = t_len // tch

    nc = bacc.Bacc("TRN2", target_bir_lowering=False, debug=False)

    # ---- DRAM I/O (host provides these layouts directly) ----
    xT = nc.dram_tensor("xT", [128, n_et, s_loc], bf16, kind="ExternalInput").ap()
    yT = nc.dram_tensor("yT", [128, n_et, t_len], bf16, kind="ExternalInput").ap()
    wq2 = nc.dram_tensor("wq2", [128, n_hp, n_et, 128], bf16, kind="ExternalInput").ap()
    wk2 = nc.dram_tensor("wk2", [128, n_hp, n_et, 128], bf16, kind="ExternalInput").ap()
    wv = nc.dram_tensor("wv", [128, n_et, c_dim], bf16, kind="ExternalInput").ap()
    wo = nc.dram_tensor("wo", [128, n_hp, e_dim], bf16, kind="ExternalInput").ap()
    bqc = nc.dram_tensor("bqc", [128, n_hp], f32, kind="ExternalInput").ap()
    bkc = nc.dram_tensor("bkc", [128, n_hp], f32, kind="ExternalInput").ap()
    bvc = nc.dram_tensor("bvc", [1, c_dim], bf16, kind="ExternalInput").ap()
    out = nc.dram_tensor("out", [s_loc, e_dim], f32, kind="ExternalOutput").ap()

    from contextlib import ExitStack

    dbg = {}
    if debug_taps:
        dbg["qt0"] = nc.dram_tensor("dbg_qt0", [128, s_loc], dt.bfloat16, kind="ExternalOutput").ap()
        dbg["kt0"] = nc.dram_tensor("dbg_kt0", [128, t_len], dt.bfloat16, kind="ExternalOutput").ap()
        dbg["v"] = nc.dram_tensor("dbg_v", [128, t_len // 128, 2 * n_hp, 65], dt.bfloat16, kind="ExternalOutput").ap()
        dbg["oT"] = nc.dram_tensor("dbg_oT", [128, n_hp, s_loc], dt.bfloat16, kind="ExternalOutput").ap()
        dbg["osc0"] = nc.dram_tensor("dbg_osc0", [64, min(512, s_loc)], dt.bfloat16, kind="ExternalOutput").ap()
        dbg["den0"] = nc.dram_tensor("dbg_den0", [1, min(512, s_loc)], f32, kind="ExternalOutput").ap()
        dbg["rcp0"] = nc.dram_tensor("dbg_rcp0", [1, min(512, s_loc)], f32, kind="ExternalOutput").ap()
        dbg["exp0"] = nc.dram_tensor("dbg_exp0", [128, 2, min(512, s_loc)], dt.bfloat16, kind="ExternalOutput").ap()

    with tile.TileContext(nc) as tc, ExitStack() as ctx:
        consts = ctx.enter_context(tc.tile_pool(name="consts", bufs=1))
        scr_pool = ctx.enter_context(tc.tile_pool(name="scr", bufs=2, space="DRAM"))
        qt_pool = ctx.enter_context(tc.tile_pool(name="qt", bufs=2))
        kt_pool = ctx.enter_context(tc.tile_pool(name="kt", bufs=2))
        exp_pool = ctx.enter_context(tc.tile_pool(name="expp", bufs=5))
        schr_pool = ctx.enter_context(tc.tile_pool(name="schr", bufs=1))
        osc_pool = ctx.enter_context(tc.tile_pool(name="osc", bufs=8))
        rcp_pool = ctx.enter_context(tc.tile_pool(name="rcp", bufs=4))
        den_pool = ctx.enter_context(tc.tile_pool(name="den", bufs=2))
        rbc_pool = ctx.enter_context(tc.tile_pool(name="rbc", bufs=4))
        osb_pool = ctx.enter_context(tc.tile_pool(name="osb", bufs=3))
        sc_ps = ctx.enter_context(tc.tile_pool(name="scps", bufs=2, space="PSUM"))
        acc_ps = ctx.enter_context(tc.tile_pool(name="accps", bufs=2, space="PSUM"))
        o_ps_pool = ctx.enter_context(tc.tile_pool(name="ops", bufs=2, space="PSUM"))

        # ---- resident tiles ----
        xT_sb = consts.tile([128, n_et, s_loc], bf16)
        yT_sb = consts.tile([128, n_et, t_len], bf16)
        wq_sb = consts.tile([128, n_hp, n_et, 128], bf16)
        wk_sb = consts.tile([128, n_hp, n_et, 128], bf16)
        wv_sb = consts.tile([128, n_et, c_dim], bf16)
        wo_sb = consts.tile([128, n_hp, e_dim], bf16)
        bqc_sb = consts.tile([128, n_hp], f32)
        bkc_sb = consts.tile([128, n_hp], f32)
        bv_sb = consts.tile([1, c_dim], bf16)

        # DMA ordering: the critical path to the first attention tile is
        # yT chunk0 -> K-proj hp0 chunk0 -> scores (plus wv for the JIT
        # V-projection). 512-col chunks keep region deps fine-grained so
        # compute starts as soon as the first column chunk of each et row
        # lands; spread across the three DMA-capable engine queues.
        bv_bc = consts.tile([128, c_dim], bf16)
        nc.sync.dma_start(out=wk_sb[:, 0, :, :], in_=wk2[:, 0, :, :])
        nc.gpsimd.dma_start(out=wq_sb[:, 0, :, :], in_=wq2[:, 0, :, :])
        # the scalar (ACT) queue gets ONLY early small transfers: every
        # dma_start on it occupies the ACT instruction FIFO, so bulk loads
        # there would stall the exp pipeline behind the DMA ring
        nc.scalar.dma_start(out=bqc_sb, in_=bqc)
        nc.scalar.dma_start(out=bkc_sb, in_=bkc)
        nc.scalar.dma_start(out=bv_sb, in_=bvc)
        # wv gates the JIT V-projection right behind the first K/Q chunks
        nc.sync.dma_start(out=wv_sb[:, :, 0 : c_dim // 2], in_=wv[:, :, 0 : c_dim // 2])
        nc.gpsimd.dma_start(out=wv_sb[:, :, c_dim // 2 :], in_=wv[:, :, c_dim // 2 :])
        nc.scalar.dma_start(out=bv_bc, in_=bvc[0:1, :].to_broadcast([128, c_dim]))
        ch = 512
        rr = [nc.sync, nc.gpsimd]
        j = 0
        # chunk 0 of y and x feeds the first K/Q projections and V tiles;
        # the remaining y chunks gate the attention tt-cursor and the JIT
        # V-projection, so they go before the remaining x chunks. The
        # scalar queue helps only with chunk 0 (done before exp starts).
        for et in range(n_et):
            cols = slice(0, ch)
            if et >= 6:
                nc.scalar.dma_start(out=yT_sb[:, et, cols], in_=yT[:, et, cols])
            else:
                rr[j % 2].dma_start(out=yT_sb[:, et, cols], in_=yT[:, et, cols])
                j += 1
        for et in range(n_et):
            cols = slice(0, ch)
            if et >= 6:
                nc.scalar.dma_start(out=xT_sb[:, et, cols], in_=xT[:, et, cols])
            else:
                rr[j % 2].dma_start(out=xT_sb[:, et, cols], in_=xT[:, et, cols])
                j += 1
        for c in range(1, t_len // ch):
            cols = slice(c * ch, (c + 1) * ch)
            for et in range(n_et):
                rr[j % 2].dma_start(out=yT_sb[:, et, cols], in_=yT[:, et, cols])
                j += 1
        for lo, hi in [(ch, 3 * ch), (3 * ch, s_loc)]:
            cols = slice(lo, hi)
            for et in range(n_et):
                rr[j % 2].dma_start(out=xT_sb[:, et, cols], in_=xT[:, et, cols])
                j += 1
        if n_hp > 1:
            nc.gpsimd.dma_start(out=wq_sb[:, 1:, :, :], in_=wq2[:, 1:, :, :])
            nc.sync.dma_start(out=wk_sb[:, 1:, :, :], in_=wk2[:, 1:, :, :])
        nc.gpsimd.dma_start(out=wo_sb, in_=wo)

        ones_row = consts.tile([1, 512], bf16)
        nc.vector.memset(ones_row, 1.0)
        # warm up the ACT exp table during the input DMA wait
        warm = consts.tile([1, 512], bf16)
        nc.scalar.activation(
            out=warm, in_=ones_row, func=_mybir().ActivationFunctionType.Exp
        )

        # V' with a ones column per head: [p, tt, head, 65]
        v_sb = consts.tile([128, n_tt, n_h, 65], bf16)
        nc.vector.memset(v_sb[:, :, :, 64:65], 1.0)
        oT_all = consts.tile([128, n_hp, s_loc], bf16)

        def v_proj_tt(tt):
            # V'[t-tile, all heads] in natural [t, c] layout; bias fused
            # into the DVE evacuation (bv broadcast tile)
            ps = acc_ps.tile([128, c_dim], f32, tag="acc")
            for et in range(n_et):
                nc.tensor.matmul(
                    out=ps,
                    lhsT=yT_sb[:, et, tt * 128 : (tt + 1) * 128],
                    rhs=wv_sb[:, et, :],
                    start=(et == 0),
                    stop=(et == n_et - 1),
                )
            nc.vector.tensor_add(
                v_sb[:, tt, :, 0:64],
                ps.rearrange("p (h d) -> p h d", d=64),
                bv_bc.rearrange("p (h d) -> p h d", d=64),
            )

        def k_chunk(hp, kt, c):
            ps = acc_ps.tile([128, tch], f32, tag="acc")
            for et in range(n_et):
                nc.tensor.matmul(
                    out=ps,
                    lhsT=wk_sb[:, hp, et, :],
                    rhs=yT_sb[:, et, c * tch : (c + 1) * tch],
                    start=(et == 0),
                    stop=(et == n_et - 1),
                )
            nc.vector.tensor_scalar_add(
                out=kt[:, c * tch : (c + 1) * tch],
                in0=ps,
                scalar1=bkc_sb[:, hp : hp + 1],
            )

        def q_chunk(hp, qt, c):
            ps = acc_ps.tile([128, qch], f32, tag="acc")
            for et in range(n_et):
                nc.tensor.matmul(
                    out=ps,
                    lhsT=wq_sb[:, hp, et, :],
                    rhs=xT_sb[:, et, c * qch : (c + 1) * qch],
                    start=(et == 0),
                    stop=(et == n_et - 1),
                )
            nc.vector.tensor_scalar_add(
                out=qt[:, c * qch : (c + 1) * qch],
                in0=ps,
                scalar1=bqc_sb[:, hp : hp + 1],
            )

        def qk_proj(hp):
            # K chunk first: it gates the first scores of qc0
            qt = qt_pool.tile([128, s_loc], bf16, tag="qt")
            kt = kt_pool.tile([128, t_len], bf16, tag="kt")
            for c in range(n_tc):
                k_chunk(hp, kt, c)
                q_chunk(hp, qt, c)
            return qt, kt

        def out_proj_unit(st, ec):
            # one partial-out tile: all heads' oT for this s-tile done
            ps = acc_ps.tile([128, ech], f32, tag="acc")
            for ct in range(n_hp):
                nc.tensor.matmul(
                    out=ps,
                    lhsT=oT_all[:, ct, st * 128 : (st + 1) * 128],
                    rhs=wo_sb[:, ct, ec * ech : (ec + 1) * ech],
                    start=(ct == 0),
                    stop=(ct == n_hp - 1),
                )
            o_sb = osb_pool.tile([128, ech], f32, tag="osb")
            nc.vector.tensor_copy(out=o_sb, in_=ps)
            nc.sync.dma_start(
                out=out[st * 128 : (st + 1) * 128, ec * ech : (ec + 1) * ech],
                in_=o_sb,
            )

        def out_proj_units(qc):
            return [
                (st, ec)
                for st in range(qc * (qch // 128), (qc + 1) * (qch // 128))
                for ec in range(n_ec)
            ]

        import math

        i32 = dt.int32
        a_schr = 0.125 * float(1 << 23) / math.log(2.0)
        b_schr = float((127 << 23) - 485000)

        def normalize(hp, qc, osc_a, osc_b, rcp_rows_dram):
            # partition-broadcast 1/sum via a DRAM round trip, then scale
            rbc_a = rbc_pool.tile([64, qch], bf16, tag="rbc")
            nc.sync.dma_start(
                out=rbc_a, in_=rcp_rows_dram[0:1, :].to_broadcast([64, qch])
            )
            rbc_b = rbc_pool.tile([64, qch], bf16, tag="rbc")
            nc.gpsimd.dma_start(
                out=rbc_b, in_=rcp_rows_dram[1:2, :].to_broadcast([64, qch])
            )
            nc.vector.tensor_mul(
                oT_all[0:64, hp, qc * qch : (qc + 1) * qch], osc_a, rbc_a
            )
            nc.vector.tensor_mul(
                oT_all[64:128, hp, qc * qch : (qc + 1) * qch], osc_b, rbc_b
            )

        pending_op = []  # out-proj units deferred into the next block
        for hp in range(n_hp):
            if hp == 0:
                # software-pipelined ramp: emit only what chunk-0 DMAs can
                # feed, then stream the rest just ahead of the tt cursor so
                # DMA-stalled matmuls never block the attention FIFO.
                qt = qt_pool.tile([128, s_loc], bf16, tag="qt")
                kt = kt_pool.tile([128, t_len], bf16, tag="kt")
                k_chunk(0, kt, 0)
                q_chunk(0, qt, 0)
                v_proj_tt(0)
                v_proj_tt(1)
            else:
                qt, kt = qk_proj(hp)
            last_hp = hp == n_hp - 1

            for qc in range(n_qc):
                o_a = o_ps_pool.tile([65, qch], f32, tag="o")
                o_b = o_ps_pool.tile([65, qch], f32, tag="o")
                for tt in range(n_tt):
                    sc_tile = sc_ps.tile([128, 2, qch], f32, tag="sc")
                    # scoresT for head a (contraction rows 0:64) and head b
                    # (rows 64:128) run concurrently via PE row tiling.
                    nc.tensor.matmul(
                        out=sc_tile[:, 0, :],
                        lhsT=kt[0:64, tt * 128 : (tt + 1) * 128],
                        rhs=qt[0:64, qc * qch : (qc + 1) * qch],
                        start=True,
                        stop=True,
                    )
                    nc.tensor.matmul(
                        out=sc_tile[:, 1, :],
                        lhsT=kt[64:128, tt * 128 : (tt + 1) * 128],
                        rhs=qt[64:128, qc * qch : (qc + 1) * qch],
                        start=True,
                        stop=True,
                    )
                    exp_t = exp_pool.tile([128, 2, qch], bf16, tag="exp")
                    if tt in dve_exp_tts:
                        # Schraudolph exp on DVE: bitcast(int(A*x + B))
                        ti = schr_pool.tile([128, 2, qch], i32, tag="ti")
                        nc.vector.tensor_scalar(
                            out=ti,
                            in0=sc_tile,
                            scalar1=a_schr,
                            scalar2=b_schr,
                            op0=_mybir().AluOpType.mult,
                            op1=_mybir().AluOpType.add,
                        )
                        nc.vector.tensor_copy(out=exp_t, in_=ti.bitcast(f32))
                    else:
                        nc.scalar.activation(
                            out=exp_t,
                            in_=sc_tile,
                            func=_mybir().ActivationFunctionType.Exp,
                            scale=0.125,
                        )
                    if debug_taps and hp == 0 and qc == 0 and tt == 0:
                        nc.sync.dma_start(out=dbg["exp0"], in_=exp_t)
                    first, last = tt == 0, tt == n_tt - 1
                    # attnV with ones column: row 64 = softmax denominator
                    nc.tensor.matmul(
                        out=o_a,
                        lhsT=v_sb[:, tt, 2 * hp, :],
                        rhs=exp_t[:, 0, :],
                        start=first,
                        stop=last,
                    )
                    nc.tensor.matmul(
                        out=o_b,
                        lhsT=v_sb[:, tt, 2 * hp + 1, :],
                        rhs=exp_t[:, 1, :],
                        start=first,
                        stop=last,
                    )
                    if hp == 0 and qc == 0:
                        # stream the remaining V tiles and K/Q chunks just
                        # ahead of where the attention will need them
                        if tt + 2 < n_tt:
                            v_proj_tt(tt + 2)
                        if tt in (1, 5, 9):
                            k_chunk(0, kt, 1 + (tt - 1) // 4)
                        if tt in (3, 7, 11):
                            q_chunk(0, qt, 1 + (tt - 3) // 4)
                    elif pending_op and (tt % 2 == 0 and tt >= 6 or tt == 15):
                        # spread the previous chunk's out-proj tiles thinly
                        # so their psum-evac latency never stalls the PE FIFO
                        out_proj_unit(*pending_op.pop(0))
                # psum evacuation: data rows to bf16, denominator rows to f32
                osc_a = osc_pool.tile([64, qch], bf16, tag="osc")
                nc.vector.tensor_copy(out=osc_a, in_=o_a[0:64, :])
                osc_b = osc_pool.tile([64, qch], bf16, tag="osc")
                nc.vector.tensor_copy(out=osc_b, in_=o_b[0:64, :])
                den_a = rcp_pool.tile([1, qch], f32, tag="den")
                nc.vector.tensor_copy(out=den_a, in_=o_a[64:65, :])
                den_b = rcp_pool.tile([1, qch], f32, tag="den")
                nc.vector.tensor_copy(out=den_b, in_=o_b[64:65, :])
                if debug_taps and hp == 0 and qc == 0:
                    nc.sync.dma_start(out=dbg["osc0"], in_=osc_a)
                    nc.sync.dma_start(out=dbg["den0"], in_=den_a)
                # gather both denominators on adjacent partitions, one
                # reciprocal call for the pair (cost is free-dim bound)
                den2 = rcp_pool.tile([2, qch], f32, tag="den2")
                nc.sync.dma_start(out=den2[0:1, :], in_=den_a)
                nc.gpsimd.dma_start(out=den2[1:2, :], in_=den_b)
                nc.vector.reciprocal(out=den2, in_=den2)
                rcp2 = rcp_pool.tile([2, qch], bf16, tag="rcp2")
                nc.vector.tensor_copy(out=rcp2, in_=den2)
                if debug_taps and hp == 0 and qc == 0:
                    nc.sync.dma_start(out=dbg["rcp0"], in_=den2[0:1, :])
                scr = scr_pool.tile([2, qch], bf16, tag="scr")
                nc.sync.dma_start(out=scr, in_=rcp2)
                normalize(hp, qc, osc_a, osc_b, scr)
                if last_hp:
                    pending_op.extend(out_proj_units(qc))
                    if qc == n_qc - 1:
                        while pending_op:
                            out_proj_unit(*pending_op.pop(0))

            if debug_taps and hp == 0:
                nc.sync.dma_start(out=dbg["qt0"], in_=qt)
                nc.sync.dma_start(out=dbg["kt0"], in_=kt)

        if debug_taps:
            nc.sync.dma_start(out=dbg["v"], in_=v_sb)
            nc.sync.dma_start(out=dbg["oT"], in_=oT_all)

    nc.compile()
    return nc


def _bf16(a):
    import ml_dtypes

    return np.ascontiguousarray(a).astype(ml_dtypes.bfloat16)


def host_prep_half(Wq, bq, Wk, bk, Wv, bv, Wo, h0, n_hp=4, n_et=8):
    """Pack one head-half's weights into the kernel's DRAM layouts."""
    e_dim = 128 * n_et
    n_heads = 2 * n_hp

    def pack_pairs(W):
        # [H, E, D] -> [p, hp, et, m] with m = j*64+d, head = h0 + 2*hp+j
        Wr = W[h0 : h0 + n_heads].reshape(n_hp, 2, e_dim, D)  # hp, j, e, d
        arr = Wr.transpose(2, 0, 1, 3).reshape(e_dim, n_hp, 128)  # e, hp, m
        arr = arr.reshape(n_et, 128, n_hp, 128).transpose(1, 2, 0, 3)
        return np.ascontiguousarray(arr)  # [p, hp, et, m]

    def bias_cols(b):
        # [H, D] -> [p, hp] with p = j*64+d
        return np.ascontiguousarray(
            b[h0 : h0 + n_heads]
            .reshape(n_hp, 2, 64)
            .transpose(1, 2, 0)
            .reshape(128, n_hp)
        ).astype(np.float32)

    c_dim = 64 * n_heads
    wv_cat = Wv[h0 : h0 + n_heads].transpose(1, 0, 2).reshape(e_dim, c_dim)  # [e, c]
    wv_arr = wv_cat.reshape(n_et, 128, c_dim).transpose(1, 0, 2)  # [p, et, c]
    wo_arr = (
        Wo[h0 * D : (h0 + n_heads) * D].reshape(n_hp, 128, e_dim).transpose(1, 0, 2)
    )  # [p, ct, e]

    return {
        "wq2": _bf16(pack_pairs(Wq)),
        "wk2": _bf16(pack_pairs(Wk)),
        "wv": _bf16(np.ascontiguousarray(wv_arr)),
        "wo": _bf16(np.ascontiguousarray(wo_arr)),
        "bqc": bias_cols(bq),
        "bkc": bias_cols(bk),
        "bvc": _bf16(bv[h0 : h0 + n_heads].reshape(1, c_dim)),
    }


def host_prep_xt(mat, n_et=8):
    """[rows, E] -> [p, et, rows] transposed tiled layout, bf16."""
    rows, e_dim = mat.shape
    assert e_dim == 128 * n_et
    arr = mat.T.reshape(n_et, 128, rows).transpose(1, 0, 2)
    return _bf16(arr)


def kernel(x, y, Wq, bq, Wk, bk, Wv, bv, Wo, bo):
    import os
    import sys

    if "/opt/trn_rl_repo" not in sys.path:
        sys.path.insert(0, "/opt/trn_rl_repo")
    from concourse import bass_utils

    x = np.asarray(x, dtype=np.float32)
    y = np.asarray(y, dtype=np.float32)

    if "prog" not in _compiled:
        _compiled["prog"] = build_program()
    nc = _compiled["prog"]

    Wq, bq = np.asarray(Wq, np.float32), np.asarray(bq, np.float32)
    Wk, bk = np.asarray(Wk, np.float32), np.asarray(bk, np.float32)
    Wv, bv = np.asarray(Wv, np.float32), np.asarray(bv, np.float32)
    Wo, bo = np.asarray(Wo, np.float32), np.asarray(bo, np.float32)

    halves = [host_prep_half(Wq, bq, Wk, bk, Wv, bv, Wo, hh * 8) for hh in range(2)]
    xT_b = [host_prep_xt(x[b]) for b in range(B)]
    yT_b = [host_prep_xt(y[b]) for b in range(B)]
    in_maps = []
    for c in range(N_CORES):
        b, hh = c // 2, c % 2
        m = dict(halves[hh])
        m["xT"] = xT_b[b]
        m["yT"] = yT_b[b]
        in_maps.append(m)

    trace = os.environ.get("TRN_ATTN_TRACE", "0") == "1"
    res = bass_utils.run_bass_kernel_spmd(
        nc, in_maps, core_ids=list(range(N_CORES)), trace=trace
    )
    _compiled["last_results"] = res
    out = np.empty((B, S, E), dtype=np.float32)
    for b in range(B):
        out[b] = res.results[2 * b]["out"]
        out[b] += res.results[2 * b + 1]["out"]
        out[b] += bo
    return out


# revision 42
# speedup vs baseline: 1.1827x; 1.0143x over previous
"""Cross multi-head attention on 8 Trainium2 NeuronCores.

Sharding: tensor-parallel over heads x data-parallel over batch.
Core c handles batch b = c//2 and head-half hh = c%2 (8 of 16 heads),
full S=2048 queries, full T=2048 keys. No K/V recompute (the old
batch x seq-half layout projected K/V twice per batch). Each core
emits a PARTIAL output (its 8 heads' concat @ Wo slice, no bias);
the host sums the two partials per batch and adds bo. That host-side
reduce is the tensor-parallel unshard for the contraction split of
combo_linear.

Per-core kernel (T-major layout, no on-chip transposes; host
pre-transposes x/y and pre-packs weights):
  QT[hp]  [128,S]  = Wq2[hp].T @ xT      (head-pair packed: rows 0:64 head a,
  KT[hp]  [128,T]  = Wk2[hp].T @ yT       rows 64:128 head b; bias fused into
                                          the PSUM->SBUF copy on DVE)
  V'      [t,h,65] = yT.T @ Wv_cat | 1   (natural layout + ones column;
                                          projected per t-tile just-in-time
                                          inside the first attention pass)
  scoresT [t,q]    = KT_h.T @ QT_h       (K=64 contraction; two heads truly
                                          concurrent via PE row tiling)
  expT    = exp(scoresT * 0.125)         (one ACT op per psum pair tile)
  oT'     += V'_h.T @ expT               (M=65: row 64 accumulates the softmax
                                          denominator for free)
  oT      = oT'[0:64] * bcast(1/oT'[64]) (deferred normalization; fast
                                          approximate reciprocal on DVE)
  partial = concatT.T @ Wo               (per-qc, interleaved into the last
                                          head-pair's attention)
Matmul inputs bf16 (fp32 PSUM accumulation), softmax in fp32.
"""

import numpy as np

B, S, T, E, H, D = 4, 2048, 2048, 1024, 16, 64
N_CORES = 8

_compiled = {}


def _dt():
    from concourse import mybir

    return mybir.dt


def _mybir():
    from concourse import mybir

    return mybir


def build_program(n_hp=4, s_loc=2048, t_len=2048, n_et=8, dve_exp_tts=(), debug_taps=False):
    """Emit the per-core bass program.

    n_hp: head pairs on this core (heads = 2*n_hp), s_loc: query rows,
    t_len: key rows, n_et: contraction tiles (emb dim = 128*n_et).
    dve_exp_tts: set of tt indices whose softmax exp runs on DVE via the
    Schraudolph bit trick instead of ACT (load balancing knob).
    """
    import concourse.tile as tile
    from concourse import bacc

    dt = _dt()
    bf16 = dt.bfloat16
    f32 = dt.float32

    e_dim = 128 * n_et
    c_dim = 64 * 2 * n_hp  # concat dim of this core's heads
    n_h = 2 * n_hp
    n_tt = t_len // 128  # key tiles
    qch = min(512, s_loc)  # query chunk width
    tch = min(512, t_len)
    ech = min(512, e_dim)
    n_qc = s_loc // qch  # query chunks for attention
    n_st = s_loc // 128  # output row tiles
    n_ec = e_dim // ech  # output col chunks
    n_tc = t_len // tch

    nc = bacc.Bacc("TRN2", target_bir_lowering=False, debug=False)

    # ---- DRAM I/O (host provides these layouts directly) ----
    xT = nc.dram_tensor("xT", [128, n_et, s_loc], bf16, kind="ExternalInput").ap()
    yT = nc.dram_tensor("yT", [128, n_et, t_len], bf16, kind="ExternalInput").ap()
    wq2 = nc.dram_tensor("wq2", [128, n_hp, n_et, 128], bf16, kind="ExternalInput").ap()
    wk2 = nc.dram_tensor("wk2", [128, n_hp, n_et, 128], bf16, kind="ExternalInput").ap()
    wv = nc.dram_tensor("wv", [128, n_et, c_dim], bf16, kind="ExternalInput").ap()
    wo = nc.dram_tensor("wo", [128, n_hp, e_dim], bf16, kind="ExternalInput").ap()
    bqc = nc.dram_tensor("bqc", [128, n_hp], f32, kind="ExternalInput").ap()
    bkc = nc.dram_tensor("bkc", [128, n_hp], f32, kind="ExternalInput").ap()
    bvc = nc.dram_tensor("bvc", [1, c_dim], bf16, kind="ExternalInput").ap()
    out = nc.dram_tensor("out", [s_loc, e_dim], f32, kind="ExternalOutput").ap()

    from contextlib import ExitStack

    dbg = {}
    if debug_taps:
        dbg["qt0"] = nc.dram_tensor("dbg_qt0", [128, s_loc], dt.bfloat16, kind="ExternalOutput").ap()
        dbg["kt0"] = nc.dram_tensor("dbg_kt0", [128, t_len], dt.bfloat16, kind="ExternalOutput").ap()
        dbg["v"] = nc.dram_tensor("dbg_v", [128, t_len // 128, 2 * n_hp, 65], dt.bfloat16, kind="ExternalOutput").ap()
        dbg["oT"] = nc.dram_tensor("dbg_oT", [128, n_hp, s_loc], dt.bfloat16, kind="ExternalOutput").ap()
        dbg["osc0"] = nc.dram_tensor("dbg_osc0", [64, min(512, s_loc)], dt.bfloat16, kind="ExternalOutput").ap()
        dbg["den0"] = nc.dram_tensor("dbg_den0", [1, min(512, s_loc)], f32, kind="ExternalOutput").ap()
        dbg["rcp0"] = nc.dram_tensor("dbg_rcp0", [1, min(512, s_loc)], f32, kind="ExternalOutput").ap()
        dbg["exp0"] = nc.dram_tensor("dbg_exp0", [128, 2, min(512, s_loc)], dt.bfloat16, kind="ExternalOutput").ap()

    with tile.TileContext(nc) as tc, ExitStack() as ctx:
        consts = ctx.enter_context(tc.tile_pool(name="consts", bufs=1))
        scr_pool = ctx.enter_context(tc.tile_pool(name="scr", bufs=2, space="DRAM"))
        qt_pool = ctx.enter_context(tc.tile_pool(name="qt", bufs=2))
        kt_pool = ctx.enter_context(tc.tile_pool(name="kt", bufs=2))
        exp_pool = ctx.enter_context(tc.tile_pool(name="expp", bufs=5))
        schr_pool = ctx.enter_context(tc.tile_pool(name="schr", bufs=1))
        osc_pool = ctx.enter_context(tc.tile_pool(name="osc", bufs=8))
        rcp_pool = ctx.enter_context(tc.tile_pool(name="rcp", bufs=4))
        den_pool = ctx.enter_context(tc.tile_pool(name="den", bufs=2))
        rbc_pool = ctx.enter_context(tc.tile_pool(name="rbc", bufs=4))
        osb_pool = ctx.enter_context(tc.tile_pool(name="osb", bufs=3))
        sc_ps = ctx.enter_context(tc.tile_pool(name="scps", bufs=2, space="PSUM"))
        acc_ps = ctx.enter_context(tc.tile_pool(name="accps", bufs=2, space="PSUM"))
        o_ps_pool = ctx.enter_context(tc.tile_pool(name="ops", bufs=2, space="PSUM"))

        # ---- resident tiles ----
        xT_sb = consts.tile([128, n_et, s_loc], bf16)
        yT_sb = consts.tile([128, n_et, t_len], bf16)
        wq_sb = consts.tile([128, n_hp, n_et, 128], bf16)
        wk_sb = consts.tile([128, n_hp, n_et, 128], bf16)
        wv_sb = consts.tile([128, n_et, c_dim], bf16)
        wo_sb = consts.tile([128, n_hp, e_dim], bf16)
        bqc_sb = consts.tile([128, n_hp], f32)
        bkc_sb = consts.tile([128, n_hp], f32)
        bv_sb = consts.tile([1, c_dim], bf16)

        # DMA ordering: the critical path to the first attention tile is
        # yT chunk0 -> K-proj hp0 chunk0 -> scores (plus wv for the JIT
        # V-projection). 512-col chunks keep region deps fine-grained so
        # compute starts as soon as the first column chunk of each et row
        # lands; spread across the three DMA-capable engine queues.
        bv_bc = consts.tile([128, c_dim], bf16)
        nc.sync.dma_start(out=wk_sb[:, 0, :, :], in_=wk2[:, 0, :, :])
        nc.gpsimd.dma_start(out=wq_sb[:, 0, :, :], in_=wq2[:, 0, :, :])
        # the scalar (ACT) queue gets ONLY early small transfers: every
        # dma_start on it occupies the ACT instruction FIFO, so bulk loads
        # there would stall the exp pipeline behind the DMA ring
        nc.scalar.dma_start(out=bqc_sb, in_=bqc)
        nc.scalar.dma_start(out=bkc_sb, in_=bkc)
        nc.scalar.dma_start(out=bv_sb, in_=bvc)
        # wv gates the JIT V-projection right behind the first K/Q chunks
        nc.sync.dma_start(out=wv_sb[:, :, 0 : c_dim // 2], in_=wv[:, :, 0 : c_dim // 2])
        nc.gpsimd.dma_start(out=wv_sb[:, :, c_dim // 2 :], in_=wv[:, :, c_dim // 2 :])
        nc.scalar.dma_start(out=bv_bc, in_=bvc[0:1, :].to_broadcast([128, c_dim]))
        ch = 512
        rr = [nc.sync, nc.gpsimd]
        j = 0
        # chunk 0 of y and x feeds the first K/Q projections and V tiles;
        # the remaining y chunks gate the attention tt-cursor and the JIT
        # V-projection, so they go before the remaining x chunks. The
        # scalar queue helps only with chunk 0 (done before exp starts).
        for et in range(n_et):
            cols = slice(0, ch)
            if et >= 6:
                nc.scalar.dma_start(out=yT_sb[:, et, cols], in_=yT[:, et, cols])
            else:
                rr[j % 2].dma_start(out=yT_sb[:, et, cols], in_=yT[:, et, cols])
                j += 1
        for et in range(n_et):
            cols = slice(0, ch)
            if et >= 6:
                nc.scalar.dma_start(out=xT_sb[:, et, cols], in_=xT[:, et, cols])
            else:
                rr[j % 2].dma_start(out=xT_sb[:, et, cols], in_=xT[:, et, cols])
                j += 1
        for c in range(1, t_len // ch):
            cols = slice(c * ch, (c + 1) * ch)
            for et in range(n_et):
                rr[j % 2].dma_start(out=yT_sb[:, et, cols], in_=yT[:, et, cols])
                j += 1
        for lo, hi in [(ch, 3 * ch), (3 * ch, s_loc)]:
            cols = slice(lo, hi)
            for et in range(n_et):
                rr[j % 2].dma_start(out=xT_sb[:, et, cols], in_=xT[:, et, cols])
                j += 1
        if n_hp > 1:
            nc.gpsimd.dma_start(out=wq_sb[:, 1:, :, :], in_=wq2[:, 1:, :, :])
            nc.sync.dma_start(out=wk_sb[:, 1:, :, :], in_=wk2[:, 1:, :, :])
        nc.gpsimd.dma_start(out=wo_sb, in_=wo)

        ones_row = consts.tile([1, 512], bf16)
        nc.vector.memset(ones_row, 1.0)
        # warm up the ACT exp table during the input DMA wait
        warm = consts.tile([1, 512], bf16)
        nc.scalar.activation(
            out=warm, in_=ones_row, func=_mybir().ActivationFunctionType.Exp
        )

        # V' with a ones column per head: [p, tt, head, 65]
        v_sb = consts.tile([128, n_tt, n_h, 65], bf16)
        nc.vector.memset(v_sb[:, :, :, 64:65], 1.0)
        oT_all = consts.tile([128, n_hp, s_loc], bf16)

        def v_proj_tt(tt):
            # V'[t-tile, all heads] in natural [t, c] layout; bias fused
            # into the DVE evacuation (bv broadcast tile)
            ps = acc_ps.tile([128, c_dim], f32, tag="acc")
            for et in range(n_et):
                nc.tensor.matmul(
                    out=ps,
                    lhsT=yT_sb[:, et, tt * 128 : (tt + 1) * 128],
                    rhs=wv_sb[:, et, :],
                    start=(et == 0),
                    stop=(et == n_et - 1),
                )
            nc.vector.tensor_add(
                v_sb[:, tt, :, 0:64],
                ps.rearrange("p (h d) -> p h d", d=64),
                bv_bc.rearrange("p (h d) -> p h d", d=64),
            )

        def k_chunk(hp, kt, c):
            ps = acc_ps.tile([128, tch], f32, tag="acc")
            for et in range(n_et):
                nc.tensor.matmul(
                    out=ps,
                    lhsT=wk_sb[:, hp, et, :],
                    rhs=yT_sb[:, et, c * tch : (c + 1) * tch],
                    start=(et == 0),
                    stop=(et == n_et - 1),
                )
            nc.vector.tensor_scalar_add(
                out=kt[:, c * tch : (c + 1) * tch],
                in0=ps,
                scalar1=bkc_sb[:, hp : hp + 1],
            )

        def q_chunk(hp, qt, c):
            ps = acc_ps.tile([128, qch], f32, tag="acc")
            for et in range(n_et):
                nc.tensor.matmul(
                    out=ps,
                    lhsT=wq_sb[:, hp, et, :],
                    rhs=xT_sb[:, et, c * qch : (c + 1) * qch],
                    start=(et == 0),
                    stop=(et == n_et - 1),
                )
            nc.vector.tensor_scalar_add(
                out=qt[:, c * qch : (c + 1) * qch],
                in0=ps,
                scalar1=bqc_sb[:, hp : hp + 1],
            )

        def qk_proj(hp):
            # K chunk first: it gates the first scores of qc0
            qt = qt_pool.tile([128, s_loc], bf16, tag="qt")
            kt = kt_pool.tile([128, t_len], bf16, tag="kt")
            for c in range(n_tc):
                k_chunk(hp, kt, c)
                q_chunk(hp, qt, c)
            return qt, kt

        def out_proj_unit(st, ec):
            # one partial-out tile: all heads' oT for this s-tile done
            ps = acc_ps.tile([128, ech], f32, tag="acc")
            for ct in range(n_hp):
                nc.tensor.matmul(
                    out=ps,
                    lhsT=oT_all[:, ct, st * 128 : (st + 1) * 128],
                    rhs=wo_sb[:, ct, ec * ech : (ec + 1) * ech],
                    start=(ct == 0),
                    stop=(ct == n_hp - 1),
                )
            o_sb = osb_pool.tile([128, ech], f32, tag="osb")
            nc.vector.tensor_copy(out=o_sb, in_=ps)
            nc.sync.dma_start(
                out=out[st * 128 : (st + 1) * 128, ec * ech : (ec + 1) * ech],
                in_=o_sb,
            )

        def out_proj_units(qc):
            return [
                (st, ec)
                for st in range(qc * (qch // 128), (qc + 1) * (qch // 128))
                for ec in range(n_ec)
            ]

        import math

        i32 = dt.int32
        a_schr = 0.125 * float(1 << 23) / math.log(2.0)
        b_schr = float((127 << 23) - 485000)

        def normalize(hp, qc, osc_a, osc_b, rcp_rows_dram):
            # partition-broadcast 1/sum via a DRAM round trip, then scale
            rbc_a = rbc_pool.tile([64, qch], bf16, tag="rbc")
            nc.sync.dma_start(
                out=rbc_a, in_=rcp_rows_dram[0:1, :].to_broadcast([64, qch])
            )
            rbc_b = rbc_pool.tile([64, qch], bf16, tag="rbc")
            nc.gpsimd.dma_start(
                out=rbc_b, in_=rcp_rows_dram[1:2, :].to_broadcast([64, qch])
            )
            nc.vector.tensor_mul(
                oT_all[0:64, hp, qc * qch : (qc + 1) * qch], osc_a, rbc_a
            )
            nc.vector.tensor_mul(
                oT_all[64:128, hp, qc * qch : (qc + 1) * qch], osc_b, rbc_b
            )

        pending_op = []  # out-proj units deferred into the next block
        for hp in range(n_hp):
            if hp == 0:
                # software-pipelined ramp: emit only what chunk-0 DMAs can
                # feed, then stream the rest just ahead of the tt cursor so
                # DMA-stalled matmuls never block the attention FIFO.
                qt = qt_pool.tile([128, s_loc], bf16, tag="qt")
                kt = kt_pool.tile([128, t_len], bf16, tag="kt")
                k_chunk(0, kt, 0)
                q_chunk(0, qt, 0)
                v_proj_tt(0)
                v_proj_tt(1)
            else:
                qt, kt = qk_proj(hp)
            last_hp = hp == n_hp - 1

            for qc in range(n_qc):
                o_a = o_ps_pool.tile([65, qch], f32, tag="o")
                o_b = o_ps_pool.tile([65, qch], f32, tag="o")
                for tt in range(n_tt):
                    sc_tile = sc_ps.tile([128, 2, qch], f32, tag="sc")
                    # scoresT for head a (contraction rows 0:64) and head b
                    # (rows 64:128) run concurrently via PE row tiling.
                    nc.tensor.matmul(
                        out=sc_tile[:, 0, :],
                        lhsT=kt[0:64, tt * 128 : (tt + 1) * 128],
                        rhs=qt[0:64, qc * qch : (qc + 1) * qch],
                        start=True,
                        stop=True,
                    )
                    nc.tensor.matmul(
                        out=sc_tile[:, 1, :],
                        lhsT=kt[64:128, tt * 128 : (tt + 1) * 128],
                        rhs=qt[64:128, qc * qch : (qc + 1) * qch],
                        start=True,
                        stop=True,
                    )
                    exp_t = exp_pool.tile([128, 2, qch], bf16, tag="exp")
                    if tt in dve_exp_tts:
                        # Schraudolph exp on DVE: bitcast(int(A*x + B))
                        ti = schr_pool.tile([128, 2, qch], i32, tag="ti")
                        nc.vector.tensor_scalar(
                            out=ti,
                            in0=sc_tile,
                            scalar1=a_schr,
                            scalar2=b_schr,
                            op0=_mybir().AluOpType.mult,
                            op1=_mybir().AluOpType.add,
                        )
                        nc.vector.tensor_copy(out=exp_t, in_=ti.bitcast(f32))
                    else:
                        nc.scalar.activation(
                            out=exp_t,
                            in_=sc_tile,
                            func=_mybir().ActivationFunctionType.Exp,
                            scale=0.125,
                        )
                    if debug_taps and hp == 0 and qc == 0 and tt == 0:
                        nc.sync.dma_start(out=dbg["exp0"], in_=exp_t)
                    first, last = tt == 0, tt == n_tt - 1
                    # attnV with ones column: row 64 = softmax denominator
                    nc.tensor.matmul(
                        out=o_a,
                        lhsT=v_sb[:, tt, 2 * hp, :],
                        rhs=exp_t[:, 0, :],
                        start=first,
                        stop=last,
                    )
                    nc.tensor.matmul(
                        out=o_b,
                        lhsT=v_sb[:, tt, 2 * hp + 1, :],
                        rhs=exp_t[:, 1, :],
                        start=first,
                        stop=last,
                    )
                    if hp == 0 and qc == 0:
                        # stream the remaining V tiles and K/Q chunks just
                        # ahead of where the attention will need them
                        if tt + 2 < n_tt:
                            v_proj_tt(tt + 2)
                        if tt in (1, 5, 9):
                            k_chunk(0, kt, 1 + (tt - 1) // 4)
                        if tt in (3, 7, 11):
                            q_chunk(0, qt, 1 + (tt - 3) // 4)
                    elif pending_op and (tt % 2 == 0 and tt >= 6 or tt == 15):
                        # spread the previous chunk's out-proj tiles thinly
                        # so their psum-evac latency never stalls the PE FIFO
                        out_proj_unit(*pending_op.pop(0))
                # psum evacuation: denominators first (they head the
                # reciprocal/broadcast latency chain), then the data rows
                den_a = rcp_pool.tile([1, qch], f32, tag="den")
                nc.vector.tensor_copy(out=den_a, in_=o_a[64:65, :])
                den_b = rcp_pool.tile([1, qch], f32, tag="den")
                nc.vector.tensor_copy(out=den_b, in_=o_b[64:65, :])
                osc_a = osc_pool.tile([64, qch], bf16, tag="osc")
                nc.vector.tensor_copy(out=osc_a, in_=o_a[0:64, :])
                osc_b = osc_pool.tile([64, qch], bf16, tag="osc")
                nc.vector.tensor_copy(out=osc_b, in_=o_b[0:64, :])
                if debug_taps and hp == 0 and qc == 0:
                    nc.sync.dma_start(out=dbg["osc0"], in_=osc_a)
                    nc.sync.dma_start(out=dbg["den0"], in_=den_a)
                # gather both denominators on adjacent partitions, one
                # reciprocal call for the pair (cost is free-dim bound)
                den2 = rcp_pool.tile([2, qch], f32, tag="den2")
                nc.sync.dma_start(out=den2[0:1, :], in_=den_a)
                nc.gpsimd.dma_start(out=den2[1:2, :], in_=den_b)
                nc.vector.reciprocal(out=den2, in_=den2)
                rcp2 = rcp_pool.tile([2, qch], bf16, tag="rcp2")
                nc.vector.tensor_copy(out=rcp2, in_=den2)
                if debug_taps and hp == 0 and qc == 0:
                    nc.sync.dma_start(out=dbg["rcp0"], in_=den2[0:1, :])
                scr = scr_pool.tile([2, qch], bf16, tag="scr")
                nc.sync.dma_start(out=scr, in_=rcp2)
                normalize(hp, qc, osc_a, osc_b, scr)
                if last_hp:
                    pending_op.extend(out_proj_units(qc))
                    if qc == n_qc - 1:
                        while pending_op:
                            out_proj_unit(*pending_op.pop(0))

            if debug_taps and hp == 0:
                nc.sync.dma_start(out=dbg["qt0"], in_=qt)
                nc.sync.dma_start(out=dbg["kt0"], in_=kt)

        if debug_taps:
            nc.sync.dma_start(out=dbg["v"], in_=v_sb)
            nc.sync.dma_start(out=dbg["oT"], in_=oT_all)

    nc.compile()
    return nc


def _bf16(a):
    import ml_dtypes

    return np.ascontiguousarray(a).astype(ml_dtypes.bfloat16)


def host_prep_half(Wq, bq, Wk, bk, Wv, bv, Wo, h0, n_hp=4, n_et=8):
    """Pack one head-half's weights into the kernel's DRAM layouts."""
    e_dim = 128 * n_et
    n_heads = 2 * n_hp

    def pack_pairs(W):
        # [H, E, D] -> [p, hp, et, m] with m = j*64+d, head = h0 + 2*hp+j
        Wr = W[h0 : h0 + n_heads].reshape(n_hp, 2, e_dim, D)  # hp, j, e, d
        arr = Wr.transpose(2, 0, 1, 3).reshape(e_dim, n_hp, 128)  # e, hp, m
        arr = arr.reshape(n_et, 128, n_hp, 128).transpose(1, 2, 0, 3)
        return np.ascontiguousarray(arr)  # [p, hp, et, m]

    def bias_cols(b):
        # [H, D] -> [p, hp] with p = j*64+d
        return np.ascontiguousarray(
            b[h0 : h0 + n_heads]
            .reshape(n_hp, 2, 64)
            .transpose(1, 2, 0)
            .reshape(128, n_hp)
        ).astype(np.float32)

    c_dim = 64 * n_heads
    wv_cat = Wv[h0 : h0 + n_heads].transpose(1, 0, 2).reshape(e_dim, c_dim)  # [e, c]
    wv_arr = wv_cat.reshape(n_et, 128, c_dim).transpose(1, 0, 2)  # [p, et, c]
    wo_arr = (
        Wo[h0 * D : (h0 + n_heads) * D].reshape(n_hp, 128, e_dim).transpose(1, 0, 2)
    )  # [p, ct, e]

    return {
        "wq2": _bf16(pack_pairs(Wq)),
        "wk2": _bf16(pack_pairs(Wk)),
        "wv": _bf16(np.ascontiguousarray(wv_arr)),
        "wo": _bf16(np.ascontiguousarray(wo_arr)),
        "bqc": bias_cols(bq),
        "bkc": bias_cols(bk),
        "bvc": _bf16(bv[h0 : h0 + n_heads].reshape(1, c_dim)),
    }


def host_prep_xt(mat, n_et=8):
    """[rows, E] -> [p, et, rows] transposed tiled layout, bf16."""
    rows, e_dim = mat.shape
    assert e_dim == 128 * n_et
    arr = mat.T.reshape(n_et, 128, rows).transpose(1, 0, 2)
    return _bf16(arr)


def kernel(x, y, Wq, bq, Wk, bk, Wv, bv, Wo, bo):
    import os
    import sys

    if "/opt/trn_rl_repo" not in sys.path:
        sys.path.insert(0, "/opt/trn_rl_repo")
    from concourse import bass_utils

    x = np.asarray(x, dtype=np.float32)
    y = np.asarray(y, dtype=np.float32)

    if "prog" not in _compiled:
        _compiled["prog"] = build_program()
    nc = _compiled["prog"]

    Wq, bq = np.asarray(Wq, np.float32), np.asarray(bq, np.float32)
    Wk, bk = np.asarray(Wk, np.float32), np.asarray(bk, np.float32)
    Wv, bv = np.asarray(Wv, np.float32), np.asarray(bv, np.float32)
    Wo, bo = np.asarray(Wo, np.float32), np.asarray(bo, np.float32)

    halves = [host_prep_half(Wq, bq, Wk, bk, Wv, bv, Wo, hh * 8) for hh in range(2)]
    xT_b = [host_prep_xt(x[b]) for b in range(B)]
    yT_b = [host_prep_xt(y[b]) for b in range(B)]
    in_maps = []
    for c in range(N_CORES):
        b, hh = c // 2, c % 2
        m = dict(halves[hh])
        m["xT"] = xT_b[b]
        m["yT"] = yT_b[b]
        in_maps.append(m)

    trace = os.environ.get("TRN_ATTN_TRACE", "0") == "1"
    res = bass_utils.run_bass_kernel_spmd(
        nc, in_maps, core_ids=list(range(N_CORES)), trace=trace
    )
    _compiled["last_results"] = res
    out = np.empty((B, S, E), dtype=np.float32)
    for b in range(B):
        out[b] = res.results[2 * b]["out"]
        out[b] += res.results[2 * b + 1]["out"]
        out[b] += bo
    return out
